# revision 27
# baseline (speedup 1.0000x reference)
"""Trainium2 Bass kernel for GQA attention (nn_Attention_15015205667492).

Reference computation (per batch b, seq s=2048, d=2048):
  q = (x @ wq)  -> 32 heads x 64     (RoPE)
  k = (x @ wk)  ->  8 kv heads x 64  (RoPE)
  v = (x @ wv)  ->  8 kv heads x 64
  causal softmax(q k^T / 8) @ v  (GQA: kv head = q head // 4)
  out = attn @ wo
Sharding (8 cores): DP2 x TP4.
  core c: batch = c//4, head-group g = c%4 (Q heads 8g..8g+7, KV heads 2g, 2g+1).

All matmuls bf16 (fp8 quantization error ~2.7%/operand does not average down
over random-sign dot products and blows the 2e-2 gate).  Layout:
  - x transposed + bf16 (xT [d, s]) so projections contract d on partitions.
  - Scores transposed per 128-key block (S^T = K^T.T @ Q^T); the kv head is
    duplicated across both 64-partition halves so a head pair's two QK
    matmuls land on PE row tiles (0,0)/(64,0) and can execute concurrently.
  - Softmax numerator exp() on ScalarE writes P^T (bf16) straight from the
    score psum; the AV matmul consumes P^T directly.  V carries a ones
    column so AV psum row 64 accumulates the denominator for free.
  - Causal masking: every masked region is a 128-col window of one shared
    lower-triangle [128,128]; diagonal blocks get a small DVE multiply
    (plus a memset for the fully-masked j=1 prefix) instead of full-block
    mask multiplies.  Blocks are emitted diagonal-first so the mask latency
    hides under the off-diagonal blocks' work.
  - PSUM drains run on VectorE; RoPE rotate-half is a PE permutation matmul;
    RoPE cos-multiply runs on gpsimd to offload DVE.
  - o_proj consumes the AllGathered [2048, 512] bf16 attention output and is
    emitted interleaved with late attention so PE fills ScalarE-bound gaps.
"""

import sys

sys.path.insert(0, "/opt/trn_rl_repo")

import numpy as np
import ml_dtypes

N_CORES = 8
H, KVH, HD = 32, 8, 64
RG = [[0, 1, 2, 3], [4, 5, 6, 7]]

_cache = {}


def build_program(S=2048, D=2048, enable_asserts=False, NO_CC=False, bench_iters=0):
    import concourse.mybir as mybir
    import concourse.tile as tile
    from concourse import bacc

    f32 = mybir.dt.float32
    bf16 = mybir.dt.bfloat16
    Exp = mybir.ActivationFunctionType.Exp

    DC = D // 128       # contraction chunks (16)
    QB = S // 512       # query blocks (4)
    KB = S // 128       # key blocks (16)
    DOUT = D // 4       # output column slice per core (512)
    HC = (H * HD) // 128  # o_proj contraction chunks (16)

    nc = bacc.Bacc(
        "TRN2",
        target_bir_lowering=False,
        debug=False,
        enable_asserts=enable_asserts,
        num_devices=N_CORES,
    )

    xT_d = nc.dram_tensor("xT", [D, S], bf16, kind="ExternalInput")
    wq_d = nc.dram_tensor("wq", [D, 512], bf16, kind="ExternalInput")
    wk_d = nc.dram_tensor("wk", [D, 128], bf16, kind="ExternalInput")
    wv_d = nc.dram_tensor("wv", [D, 128], bf16, kind="ExternalInput")
    wo_d = nc.dram_tensor("wo", [H * HD, DOUT], bf16, kind="ExternalInput")
    cos_d = nc.dram_tensor("cos2", [128, S], bf16, kind="ExternalInput")
    sin_d = nc.dram_tensor("sinsw2", [128, S], bf16, kind="ExternalInput")
    rot_d = nc.dram_tensor("rot", [128, 128], bf16, kind="ExternalInput")
    mtri_d = nc.dram_tensor("mtri", [128, 2, 128], bf16, kind="ExternalInput")
    out_d = nc.dram_tensor("out", [S, DOUT], f32, kind="ExternalOutput")

    with tile.TileContext(nc) as tc:
        with (
            tc.tile_pool(name="const", bufs=1) as const,
            tc.tile_pool(name="psA", bufs=2, space="PSUM") as psA,
            tc.tile_pool(name="psAV", bufs=1, space="PSUM") as psAV,
            tc.tile_pool(name="psP", bufs=1, space="PSUM") as psP,
            tc.tile_pool(name="work", bufs=2) as work,
            tc.tile_pool(name="dram", bufs=1, space="DRAM") as dram,
        ):
            # ------------- constant DMAs (issue order matters) -------------
            wk_t, wv_t = [], []
            for i in range(DC):
                t = const.tile([128, 128], bf16, name=f"wk{i}", tag=f"wk{i}")
                nc.sync.dma_start(out=t[:], in_=wk_d[128 * i : 128 * (i + 1), :])
                wk_t.append(t)
                t = const.tile([128, 128], bf16, name=f"wv{i}", tag=f"wv{i}")
                nc.sync.dma_start(out=t[:], in_=wv_d[128 * i : 128 * (i + 1), :])
                wv_t.append(t)
            rot_sb = const.tile([128, 128], bf16, name="rot", tag="rot")
            nc.sync.dma_start(out=rot_sb[:], in_=rot_d[:, :])
            mtri_sb = const.tile([128, 2, 128], bf16, name="mtri", tag="mtri")
            nc.sync.dma_start(out=mtri_sb[:], in_=mtri_d[:, :, :])
            xt = []
            for i in range(DC):
                t = const.tile([128, S], bf16, name=f"xt{i}", tag=f"xt{i}")
                nc.sync.dma_start(out=t[:], in_=xT_d[128 * i : 128 * (i + 1), :])
                xt.append(t)
            cos_sb = const.tile([128, S], bf16, name="cos", tag="cos")
            nc.sync.dma_start(out=cos_sb[:], in_=cos_d[:, :])
            sin_sb = const.tile([128, S], bf16, name="sin", tag="sin")
            nc.sync.dma_start(out=sin_sb[:], in_=sin_d[:, :])
            wq_t = []
            for i in range(DC):
                t = const.tile([128, 512], bf16, name=f"wq{i}", tag=f"wq{i}")
                nc.sync.dma_start(out=t[:], in_=wq_d[128 * i : 128 * (i + 1), :])
                wq_t.append(t)
            wo_t = []
            for i in range(HC):
                t = const.tile([128, DOUT], bf16, name=f"wo{i}", tag=f"wo{i}")
                nc.sync.dma_start(out=t[:], in_=wo_d[128 * i : 128 * (i + 1), :])
                wo_t.append(t)
            ones_sb = const.tile([65, 64], bf16, name="ones", tag="ones")
            nc.vector.memset(ones_sb[:], 1.0)

            def emit_body():
                # ------------- Q/K projection + RoPE -------------
                def proj_rope_gen(w_tiles, col0, dest, c2):
                    # one 1024-col seq chunk: project + RoPE into dest.
                    # Yields between small emission units so the chunks can be
                    # interleaved ("pumped") between attention blocks.
                    raw = work.tile([128, 1024], bf16, name="raw", tag="raw", bufs=2)
                    tmp = work.tile([128, 1024], bf16, name="ropetmp", tag="ropetmp", bufs=2)
                    for q2 in range(2):
                        qc = 2 * c2 + q2
                        pq = psP.tile([128, 512], f32, name="pq", tag="fill")
                        for dc0 in range(0, DC, 4):
                            for dc in range(dc0, dc0 + 4):
                                nc.tensor.matmul(
                                    pq[:],
                                    w_tiles[dc][:, col0 : col0 + 128],
                                    xt[dc][:, 512 * qc : 512 * (qc + 1)],
                                    start=(dc == 0),
                                    stop=(dc == DC - 1),
                                )
                            yield
                        nc.vector.tensor_copy(
                            out=raw[:, 512 * q2 : 512 * (q2 + 1)], in_=pq[:]
                        )
                        yield
                    # rotate-half via PE permutation, sign folded into sinsw2
                    for q2 in range(2):
                        pr = psP.tile([128, 512], f32, name="pr", tag="fill")
                        nc.tensor.matmul(
                            pr[:],
                            rot_sb[:],
                            raw[:, 512 * q2 : 512 * (q2 + 1)],
                            start=True,
                            stop=True,
                        )
                        nc.vector.tensor_mul(
                            tmp[:, 512 * q2 : 512 * (q2 + 1)],
                            pr[:],
                            sin_sb[:, 1024 * c2 + 512 * q2 : 1024 * c2 + 512 * (q2 + 1)],
                        )
                        yield
                    nc.gpsimd.tensor_mul(
                        raw[:], raw[:], cos_sb[:, 1024 * c2 : 1024 * (c2 + 1)]
                    )
                    nc.vector.tensor_add(
                        dest[:, 1024 * c2 : 1024 * (c2 + 1)], raw[:], tmp[:]
                    )
                    yield

                def run_gen(g):
                    for _ in g:
                        pass

                qT = [
                    const.tile([128, S], bf16, name=f"qT{p}", tag=f"qT{p}")
                    for p in range(4)
                ]
                krope = work.tile([128, S], bf16, name="krope", tag="krope", bufs=1)
                for c2 in range(2):
                    run_gen(proj_rope_gen(wk_t, 0, krope, c2))
                # duplicate each kv head across both 64-partition halves
                kTd = []
                for h in range(2):
                    t = const.tile([128, S], bf16, name=f"kTd{h}", tag=f"kTd{h}")
                    nc.sync.dma_start(out=t[0:64, :], in_=krope[64 * h : 64 * h + 64, :])
                    nc.sync.dma_start(out=t[64:128, :], in_=krope[64 * h : 64 * h + 64, :])
                    kTd.append(t)

                # ------------- V projection (natural, +ones cols) ----------
                v_sb = []
                for kb in range(KB):
                    vt = const.tile([128, 132], bf16, name=f"v{kb}", tag=f"v{kb}")
                    nc.vector.memset(vt[:, 64:65], 1.0)
                    nc.vector.memset(vt[:, 129:130], 1.0)
                    pv = psP.tile([128, 512], f32, name="pv", tag="fill")
                    for dc in range(DC):
                        nc.tensor.matmul(
                            pv[:, 0:128],
                            xt[dc][:, 128 * kb : 128 * (kb + 1)],
                            wv_t[dc][:],
                            start=(dc == 0),
                            stop=(dc == DC - 1),
                        )
                    nc.vector.tensor_copy(out=vt[:, 0:64], in_=pv[:, 0:64])
                    nc.vector.tensor_copy(out=vt[:, 65:129], in_=pv[:, 64:128])
                    v_sb.append(vt)

                # ------------- attention + AllGather + o_proj -------------
                cc_in = [
                    dram.tile([512, 512], bf16, name=f"cin{qb}", tag=f"cin{qb}")
                    for qb in range(QB)
                ]
                cc_out = [
                    dram.tile([H * HD, 512], bf16, name=f"cout{qb}", tag=f"cout{qb}")
                    for qb in range(QB)
                ]

                def attn_emit(qb, fill=None):
                    def pump(n=1):
                        if fill is not None:
                            for _ in range(n):
                                if next(fill, "done") == "done":
                                    break

                    kmax = 4 * (qb + 1)
                    # diagonal blocks first: their exp+mask latency hides
                    # under the off-diagonal blocks' QK/AV work
                    order = [4 * qb, 4 * qb + 1, 4 * qb + 2, 4 * qb + 3] + list(
                        range(4 * qb)
                    )
                    for hg in range(2):      # kv head (local)
                        for p2 in range(2):  # head pair within kv group
                            pidx = 2 * hg + p2
                            pav = psAV.tile([65, 1024], f32, name="pav", tag="pav")
                            for ki, kb in enumerate(order):
                                j = kb - 4 * qb
                                vw = 512 - 128 * j if j >= 2 else 512
                                q0 = 512 * qb + (512 - vw)
                                ps = psA.tile([128, 1024], f32, name="ps", tag="ps")
                                for i in range(2):
                                    r0 = 64 * i
                                    nc.tensor.matmul(
                                        ps[:, 512 * i : 512 * i + vw],
                                        kTd[hg][r0 : r0 + 64, 128 * kb : 128 * (kb + 1)],
                                        qT[pidx][r0 : r0 + 64, q0 : q0 + vw],
                                        start=True,
                                        stop=True,
                                    )
                                pt = work.tile(
                                    [128, 1024], bf16, name="pt", tag="pt", bufs=4
                                )
                                ps3 = ps.rearrange("p (i n) -> p i n", i=2)
                                p3 = pt.rearrange("p (i n) -> p i n", i=2)
                                e0 = 128 if j == 1 else 0
                                nc.scalar.activation(
                                    out=p3[:, :, e0:vw],
                                    in_=ps3[:, :, e0:vw],
                                    func=Exp,
                                    scale=0.125,
                                )
                                # causal masking: all masked regions are
                                # 128-col windows of one shared triangle
                                if j == 0 or j == 2 or j == 3:
                                    nc.vector.tensor_mul(
                                        p3[:, :, 0:128], p3[:, :, 0:128], mtri_sb[:]
                                    )
                                elif j == 1:
                                    nc.vector.memset(p3[:, :, 0:128], 0.0)
                                    nc.vector.tensor_mul(
                                        p3[:, :, 128:256], p3[:, :, 128:256], mtri_sb[:]
                                    )
                                for i in range(2):
                                    nc.tensor.matmul(
                                        pav[:, 512 * i + 512 - vw : 512 * (i + 1)],
                                        v_sb[kb][:, 65 * hg : 65 * hg + 65],
                                        pt[:, 512 * i : 512 * i + vw],
                                        start=(ki == 0),
                                        stop=(ki == kmax - 1),
                                    )
                                pump(1)
                            # normalize: out = O^T_unnorm * (1/colsum); the
                            # denominator broadcast runs on gpsimd so PE and
                            # the psP ring stay out of the group tail
                            ou = work.tile([65, 1024], bf16, name="ou", tag="ou", bufs=2)
                            nc.vector.tensor_copy(out=ou[:], in_=pav[:])
                            rbc = work.tile([64, 1024], f32, name="rbc", tag="rbc", bufs=1)
                            for i in range(2):
                                pb = psP.tile([64, 512], f32, name=f"pb{i}", tag="pb")
                                nc.tensor.matmul(
                                    pb[:],
                                    ones_sb[64:65, :],
                                    ou[64:65, 512 * i : 512 * (i + 1)],
                                    start=True,
                                    stop=True,
                                )
                                nc.vector.reciprocal_approx_fast(
                                    out=rbc[:, 512 * i : 512 * (i + 1)], in_=pb[:]
                                )
                            at = work.tile([64, 1024], bf16, name="at", tag="at")
                            nc.vector.tensor_mul(at[:], ou[0:64, :], rbc[:])
                            pump(3)
                            for i in range(2):
                                nc.sync.dma_start(
                                    out=cc_in[qb][
                                        128 * pidx + 64 * i : 128 * pidx + 64 * (i + 1), :
                                    ],
                                    in_=at[:, 512 * i : 512 * (i + 1)],
                                )
                    if NO_CC:
                        nc.sync.dma_start(out=cc_out[qb][0:512, :], in_=cc_in[qb][:, :])
                    else:
                        nc.gpsimd.collective_compute(
                            "AllGather",
                            mybir.AluOpType.bypass,
                            replica_groups=RG,
                            ins=[cc_in[qb].opt()],
                            outs=[cc_out[qb].opt()],
                        )

                def oproj_load(qb):
                    cts = []
                    for hc in range(HC):
                        t = work.tile(
                            [128, 512], bf16, name="cct", tag="cct", bufs=HC
                        )
                        nc.sync.dma_start(
                            out=t[:], in_=cc_out[qb][128 * hc : 128 * (hc + 1), :]
                        )
                        cts.append(t)
                    return cts

                def oproj_gen(qb, cts):
                    for rb in range(4):
                        po = psP.tile([128, DOUT], f32, name="po", tag="fill")
                        for hc0 in range(0, HC, 4):
                            for hc in range(hc0, hc0 + 4):
                                nc.tensor.matmul(
                                    po[:],
                                    cts[hc][:, 128 * rb : 128 * (rb + 1)],
                                    wo_t[hc][:],
                                    start=(hc == 0),
                                    stop=(hc == HC - 1),
                                )
                            yield
                        ot = work.tile([128, DOUT], f32, name="ot", tag="ot", bufs=2)
                        nc.vector.tensor_copy(out=ot[:], in_=po[:])
                        nc.sync.dma_start(
                            out=out_d[
                                512 * qb + 128 * rb : 512 * qb + 128 * (rb + 1), :
                            ],
                            in_=ot[:],
                        )
                        yield

                def chain_gens(*gens):
                    for g in gens:
                        yield from g

                # head: Q proj for the first seq half
                for p in range(4):
                    run_gen(proj_rope_gen(wq_t, 128 * p, qT[p], 0))
                # Q proj for the second half is pumped between qb0/qb1 blocks
                qproj1 = chain_gens(
                    *[proj_rope_gen(wq_t, 128 * p, qT[p], 1) for p in range(4)]
                )
                attn_emit(0, fill=qproj1)
                attn_emit(1, fill=qproj1)
                run_gen(qproj1)  # flush: qb2 needs the full qT
                og0 = oproj_gen(0, oproj_load(0))
                attn_emit(2, fill=og0)
                run_gen(og0)
                cts1 = oproj_load(1)
                cts2 = oproj_load(2)
                og12 = chain_gens(oproj_gen(1, cts1), oproj_gen(2, cts2))
                attn_emit(3, fill=og12)
                run_gen(og12)
                run_gen(oproj_gen(3, oproj_load(3)))

            if bench_iters:
                with tc.For_i(0, bench_iters, 1, name="bench"):
                    emit_body()
            else:
                emit_body()

    nc.compile()
    return nc


def prep_inputs(x, cos, sin, wq, wk, wv, wo):
    """Shard + reformat full inputs into per-core input maps."""
    bf = ml_dtypes.bfloat16
    b, s, d = x.shape
    dout = d // 4
    cos2 = np.tile(np.ascontiguousarray(cos.T), (2, 1)).astype(bf)
    sinT = np.ascontiguousarray(sin.T)
    sinsw = np.concatenate([-sinT[:32], sinT[32:]], axis=0)
    sinsw2 = np.tile(sinsw, (2, 1)).astype(bf)
    # rotate-half permutation: out = R.T @ raw
    rotm = np.zeros((128, 128), np.float32)
    for i in range(128):
        j = (i // 64) * 64 + ((i % 64) + 32) % 64
        rotm[j, i] = 1.0
    rotm = rotm.astype(bf)

    k_loc = np.arange(128)[:, None]
    c_loc = np.arange(128)[None, :]
    mtri = (k_loc <= c_loc).astype(np.float32)
    mtri2 = np.stack([mtri, mtri], axis=1).astype(bf)  # [128, 2, 128]

    in_maps = []
    for c in range(N_CORES):
        bb, g = divmod(c, 4)
        in_maps.append(
            {
                "xT": np.ascontiguousarray(x[bb].T).astype(bf),
                "wq": np.ascontiguousarray(wq[:, 512 * g : 512 * (g + 1)]).astype(bf),
                "wk": np.ascontiguousarray(wk[:, 128 * g : 128 * (g + 1)]).astype(bf),
                "wv": np.ascontiguousarray(wv[:, 128 * g : 128 * (g + 1)]).astype(bf),
                "wo": np.ascontiguousarray(wo[:, dout * g : dout * (g + 1)]).astype(bf),
                "cos2": cos2,
                "sinsw2": sinsw2,
                "rot": rotm,
                "mtri": mtri2,
            }
        )
    return in_maps


def assemble_output(results, b, s, d):
    full = np.empty((b, s, d), np.float32)
    dout = d // 4
    for c in range(N_CORES):
        bb, g = divmod(c, 4)
        full[bb][:, dout * g : dout * (g + 1)] = results[c]["out"]
    return full


def kernel(**inputs):
    x = np.asarray(inputs["x"], np.float32)
    b, s, d = x.shape
    key = (s, d)
    if key not in _cache:
        _cache[key] = build_program(S=s, D=d)
    nc = _cache[key]
    in_maps = prep_inputs(
        x,
        np.asarray(inputs["cos"], np.float32),
        np.asarray(inputs["sin"], np.float32),
        np.asarray(inputs["wq"], np.float32),
        np.asarray(inputs["wk"], np.float32),
        np.asarray(inputs["wv"], np.float32),
        np.asarray(inputs["wo"], np.float32),
    )
    from concourse.bass_utils import run_bass_kernel_spmd

    res = run_bass_kernel_spmd(nc, in_maps, core_ids=list(range(N_CORES)))
    return assemble_output(res.results, b, s, d)


# revision 28
# speedup vs baseline: 1.0632x; 1.0632x over previous
"""Trainium2 Bass kernel for GQA attention (nn_Attention_15015205667492).

Reference computation (per batch b, seq s=2048, d=2048):
  q = (x @ wq)  -> 32 heads x 64     (RoPE)
  k = (x @ wk)  ->  8 kv heads x 64  (RoPE)
  v = (x @ wv)  ->  8 kv heads x 64
  causal softmax(q k^T / 8) @ v  (GQA: kv head = q head // 4)
  out = attn @ wo
Sharding (8 cores): DP2 x TP4.
  core c: batch = c//4, head-group g = c%4 (Q heads 8g..8g+7, KV heads 2g, 2g+1).

All matmuls bf16 (fp8 quantization error ~2.7%/operand does not average down
over random-sign dot products and blows the 2e-2 gate).  Layout:
  - x transposed + bf16 (xT [d, s]) so projections contract d on partitions.
  - Scores transposed per 128-key block (S^T = K^T.T @ Q^T); the kv head is
    duplicated across both 64-partition halves so a head pair's two QK
    matmuls land on PE row tiles (0,0)/(64,0) and can execute concurrently.
  - Softmax numerator exp() on ScalarE writes P^T (bf16) straight from the
    score psum; the AV matmul consumes P^T directly.  V carries a ones
    column so AV psum row 64 accumulates the denominator for free.
  - Causal masking: every masked region is a 128-col window of one shared
    lower-triangle [128,128]; diagonal blocks get a small DVE multiply
    (plus a memset for the fully-masked j=1 prefix) instead of full-block
    mask multiplies.  Blocks are emitted diagonal-first so the mask latency
    hides under the off-diagonal blocks' work.
  - PSUM drains run on VectorE; RoPE rotate-half is a PE permutation matmul;
    RoPE cos-multiply runs on gpsimd to offload DVE.
  - o_proj consumes the AllGathered [2048, 512] bf16 attention output and is
    emitted interleaved with late attention so PE fills ScalarE-bound gaps.
"""

import sys

sys.path.insert(0, "/opt/trn_rl_repo")

import numpy as np
import ml_dtypes

N_CORES = 8
H, KVH, HD = 32, 8, 64
RG = [[0, 1, 2, 3], [4, 5, 6, 7]]

_cache = {}


def build_program(S=2048, D=2048, enable_asserts=False, NO_CC=False, bench_iters=0):
    import concourse.mybir as mybir
    import concourse.tile as tile
    from concourse import bacc

    f32 = mybir.dt.float32
    bf16 = mybir.dt.bfloat16
    Exp = mybir.ActivationFunctionType.Exp

    DC = D // 128       # contraction chunks (16)
    QB = S // 512       # query blocks (4)
    KB = S // 128       # key blocks (16)
    DOUT = D // 4       # output column slice per core (512)
    HC = (H * HD) // 128  # o_proj contraction chunks (16)

    nc = bacc.Bacc(
        "TRN2",
        target_bir_lowering=False,
        debug=False,
        enable_asserts=enable_asserts,
        num_devices=N_CORES,
    )

    xT_d = nc.dram_tensor("xT", [D, S], bf16, kind="ExternalInput")
    wq_d = nc.dram_tensor("wq", [D, 512], bf16, kind="ExternalInput")
    wk_d = nc.dram_tensor("wk", [D, 128], bf16, kind="ExternalInput")
    wv_d = nc.dram_tensor("wv", [D, 128], bf16, kind="ExternalInput")
    wo_d = nc.dram_tensor("wo", [H * HD, DOUT], bf16, kind="ExternalInput")
    cos_d = nc.dram_tensor("cos2", [128, S], bf16, kind="ExternalInput")
    sin_d = nc.dram_tensor("sinsw2", [128, S], bf16, kind="ExternalInput")
    rot_d = nc.dram_tensor("rot", [128, 128], bf16, kind="ExternalInput")
    mtri_d = nc.dram_tensor("mtri", [128, 2, 128], bf16, kind="ExternalInput")
    out_d = nc.dram_tensor("out", [S, DOUT], f32, kind="ExternalOutput")

    with tile.TileContext(nc) as tc:
        with (
            tc.tile_pool(name="const", bufs=1) as const,
            tc.tile_pool(name="psA", bufs=2, space="PSUM") as psA,
            tc.tile_pool(name="psAV", bufs=1, space="PSUM") as psAV,
            tc.tile_pool(name="psP", bufs=2, space="PSUM") as psP,
            tc.tile_pool(name="work", bufs=2) as work,
            tc.tile_pool(name="dram", bufs=1, space="DRAM") as dram,
        ):
            # ------------- constant DMAs (issue order matters) -------------
            wk_t, wv_t = [], []
            for i in range(DC):
                t = const.tile([128, 128], bf16, name=f"wk{i}", tag=f"wk{i}")
                nc.sync.dma_start(out=t[:], in_=wk_d[128 * i : 128 * (i + 1), :])
                wk_t.append(t)
                t = const.tile([128, 128], bf16, name=f"wv{i}", tag=f"wv{i}")
                nc.sync.dma_start(out=t[:], in_=wv_d[128 * i : 128 * (i + 1), :])
                wv_t.append(t)
            rot_sb = const.tile([128, 128], bf16, name="rot", tag="rot")
            nc.sync.dma_start(out=rot_sb[:], in_=rot_d[:, :])
            mtri_sb = const.tile([128, 2, 128], bf16, name="mtri", tag="mtri")
            nc.sync.dma_start(out=mtri_sb[:], in_=mtri_d[:, :, :])
            xt = []
            for i in range(DC):
                t = const.tile([128, S], bf16, name=f"xt{i}", tag=f"xt{i}")
                nc.sync.dma_start(out=t[:], in_=xT_d[128 * i : 128 * (i + 1), :])
                xt.append(t)
            cos_sb = const.tile([128, S], bf16, name="cos", tag="cos")
            nc.sync.dma_start(out=cos_sb[:], in_=cos_d[:, :])
            sin_sb = const.tile([128, S], bf16, name="sin", tag="sin")
            nc.sync.dma_start(out=sin_sb[:], in_=sin_d[:, :])
            wq_t = []
            for i in range(DC):
                t = const.tile([128, 512], bf16, name=f"wq{i}", tag=f"wq{i}")
                nc.sync.dma_start(out=t[:], in_=wq_d[128 * i : 128 * (i + 1), :])
                wq_t.append(t)
            wo_t = []
            for i in range(HC):
                t = const.tile([128, DOUT], bf16, name=f"wo{i}", tag=f"wo{i}")
                nc.sync.dma_start(out=t[:], in_=wo_d[128 * i : 128 * (i + 1), :])
                wo_t.append(t)
            ones_sb = const.tile([65, 64], bf16, name="ones", tag="ones")
            nc.vector.memset(ones_sb[:], 1.0)

            def emit_body():
                # ------------- Q/K projection + RoPE -------------
                def proj_rope_gen(w_tiles, col0, dest, c2):
                    # one 1024-col seq chunk: project + RoPE into dest.
                    # Yields between small emission units so the chunks can be
                    # interleaved ("pumped") between attention blocks.
                    raw = work.tile([128, 1024], bf16, name="raw", tag="raw", bufs=2)
                    tmp = work.tile([128, 1024], bf16, name="ropetmp", tag="ropetmp", bufs=2)
                    for q2 in range(2):
                        qc = 2 * c2 + q2
                        pq = psP.tile([128, 512], f32, name="pq", tag="fill")
                        for dc0 in range(0, DC, 4):
                            for dc in range(dc0, dc0 + 4):
                                nc.tensor.matmul(
                                    pq[:],
                                    w_tiles[dc][:, col0 : col0 + 128],
                                    xt[dc][:, 512 * qc : 512 * (qc + 1)],
                                    start=(dc == 0),
                                    stop=(dc == DC - 1),
                                )
                            yield
                        nc.vector.tensor_copy(
                            out=raw[:, 512 * q2 : 512 * (q2 + 1)], in_=pq[:]
                        )
                        yield
                    # rotate-half via PE permutation, sign folded into sinsw2
                    for q2 in range(2):
                        pr = psP.tile([128, 512], f32, name="pr", tag="fill")
                        nc.tensor.matmul(
                            pr[:],
                            rot_sb[:],
                            raw[:, 512 * q2 : 512 * (q2 + 1)],
                            start=True,
                            stop=True,
                        )
                        nc.vector.tensor_mul(
                            tmp[:, 512 * q2 : 512 * (q2 + 1)],
                            pr[:],
                            sin_sb[:, 1024 * c2 + 512 * q2 : 1024 * c2 + 512 * (q2 + 1)],
                        )
                        yield
                    nc.gpsimd.tensor_mul(
                        raw[:], raw[:], cos_sb[:, 1024 * c2 : 1024 * (c2 + 1)]
                    )
                    nc.vector.tensor_add(
                        dest[:, 1024 * c2 : 1024 * (c2 + 1)], raw[:], tmp[:]
                    )
                    yield

                def run_gen(g):
                    for _ in g:
                        pass

                qT = [
                    const.tile([128, S], bf16, name=f"qT{p}", tag=f"qT{p}")
                    for p in range(4)
                ]
                krope = work.tile([128, S], bf16, name="krope", tag="krope", bufs=1)
                for c2 in range(2):
                    run_gen(proj_rope_gen(wk_t, 0, krope, c2))
                # duplicate each kv head across both 64-partition halves
                kTd = []
                for h in range(2):
                    t = const.tile([128, S], bf16, name=f"kTd{h}", tag=f"kTd{h}")
                    nc.sync.dma_start(out=t[0:64, :], in_=krope[64 * h : 64 * h + 64, :])
                    nc.sync.dma_start(out=t[64:128, :], in_=krope[64 * h : 64 * h + 64, :])
                    kTd.append(t)

                # ------------- V projection (natural, +ones cols) ----------
                v_sb = []
                for kb in range(KB):
                    vt = const.tile([128, 132], bf16, name=f"v{kb}", tag=f"v{kb}")
                    nc.vector.memset(vt[:, 64:65], 1.0)
                    nc.vector.memset(vt[:, 129:130], 1.0)
                    pv = psP.tile([128, 512], f32, name="pv", tag="fill")
                    for dc in range(DC):
                        nc.tensor.matmul(
                            pv[:, 0:128],
                            xt[dc][:, 128 * kb : 128 * (kb + 1)],
                            wv_t[dc][:],
                            start=(dc == 0),
                            stop=(dc == DC - 1),
                        )
                    nc.vector.tensor_copy(out=vt[:, 0:64], in_=pv[:, 0:64])
                    nc.vector.tensor_copy(out=vt[:, 65:129], in_=pv[:, 64:128])
                    v_sb.append(vt)

                # ------------- attention + AllGather + o_proj -------------
                cc_in = [
                    dram.tile([512, 512], bf16, name=f"cin{qb}", tag=f"cin{qb}")
                    for qb in range(QB)
                ]
                cc_out = [
                    dram.tile([H * HD, 512], bf16, name=f"cout{qb}", tag=f"cout{qb}")
                    for qb in range(QB)
                ]

                def attn_emit(qb, fill=None):
                    def pump(n=1):
                        if fill is not None:
                            for _ in range(n):
                                if next(fill, "done") == "done":
                                    break

                    kmax = 4 * (qb + 1)
                    # diagonal blocks first: their exp+mask latency hides
                    # under the off-diagonal blocks' QK/AV work
                    order = [4 * qb, 4 * qb + 1, 4 * qb + 2, 4 * qb + 3] + list(
                        range(4 * qb)
                    )
                    for hg in range(2):      # kv head (local)
                        for p2 in range(2):  # head pair within kv group
                            pidx = 2 * hg + p2
                            pav = psAV.tile([65, 1024], f32, name="pav", tag="pav")
                            for ki, kb in enumerate(order):
                                j = kb - 4 * qb
                                vw = 512 - 128 * j if j >= 2 else 512
                                q0 = 512 * qb + (512 - vw)
                                ps = psA.tile([128, 1024], f32, name="ps", tag="ps")
                                for i in range(2):
                                    r0 = 64 * i
                                    nc.tensor.matmul(
                                        ps[:, 512 * i : 512 * i + vw],
                                        kTd[hg][r0 : r0 + 64, 128 * kb : 128 * (kb + 1)],
                                        qT[pidx][r0 : r0 + 64, q0 : q0 + vw],
                                        start=True,
                                        stop=True,
                                    )
                                pt = work.tile(
                                    [128, 1024], bf16, name="pt", tag="pt", bufs=4
                                )
                                ps3 = ps.rearrange("p (i n) -> p i n", i=2)
                                p3 = pt.rearrange("p (i n) -> p i n", i=2)
                                e0 = 128 if j == 1 else 0
                                nc.scalar.activation(
                                    out=p3[:, :, e0:vw],
                                    in_=ps3[:, :, e0:vw],
                                    func=Exp,
                                    scale=0.125,
                                )
                                # causal masking: all masked regions are
                                # 128-col windows of one shared triangle
                                if j == 0 or j == 2 or j == 3:
                                    nc.vector.tensor_mul(
                                        p3[:, :, 0:128], p3[:, :, 0:128], mtri_sb[:]
                                    )
                                elif j == 1:
                                    nc.vector.memset(p3[:, :, 0:128], 0.0)
                                    nc.vector.tensor_mul(
                                        p3[:, :, 128:256], p3[:, :, 128:256], mtri_sb[:]
                                    )
                                for i in range(2):
                                    nc.tensor.matmul(
                                        pav[:, 512 * i + 512 - vw : 512 * (i + 1)],
                                        v_sb[kb][:, 65 * hg : 65 * hg + 65],
                                        pt[:, 512 * i : 512 * i + vw],
                                        start=(ki == 0),
                                        stop=(ki == kmax - 1),
                                    )
                                pump(1)
                            # normalize: out = O^T_unnorm * (1/colsum); the
                            # denominator broadcast runs on gpsimd so PE and
                            # the psP ring stay out of the group tail
                            ou = work.tile([65, 1024], bf16, name="ou", tag="ou", bufs=2)
                            nc.vector.tensor_copy(out=ou[:], in_=pav[:])
                            rbc = work.tile([64, 1024], f32, name="rbc", tag="rbc", bufs=1)
                            for i in range(2):
                                pb = psP.tile([64, 512], f32, name=f"pb{i}", tag="fill")
                                nc.tensor.matmul(
                                    pb[:],
                                    ones_sb[64:65, :],
                                    ou[64:65, 512 * i : 512 * (i + 1)],
                                    start=True,
                                    stop=True,
                                )
                                nc.vector.reciprocal_approx_fast(
                                    out=rbc[:, 512 * i : 512 * (i + 1)], in_=pb[:]
                                )
                            at = work.tile([64, 1024], bf16, name="at", tag="at")
                            nc.vector.tensor_mul(at[:], ou[0:64, :], rbc[:])
                            pump(3)
                            for i in range(2):
                                nc.sync.dma_start(
                                    out=cc_in[qb][
                                        128 * pidx + 64 * i : 128 * pidx + 64 * (i + 1), :
                                    ],
                                    in_=at[:, 512 * i : 512 * (i + 1)],
                                )
                    if NO_CC:
                        nc.sync.dma_start(out=cc_out[qb][0:512, :], in_=cc_in[qb][:, :])
                    else:
                        nc.gpsimd.collective_compute(
                            "AllGather",
                            mybir.AluOpType.bypass,
                            replica_groups=RG,
                            ins=[cc_in[qb].opt()],
                            outs=[cc_out[qb].opt()],
                        )

                def oproj_load(qb):
                    cts = []
                    for hc in range(HC):
                        t = work.tile(
                            [128, 512], bf16, name="cct", tag="cct", bufs=HC
                        )
                        nc.sync.dma_start(
                            out=t[:], in_=cc_out[qb][128 * hc : 128 * (hc + 1), :]
                        )
                        cts.append(t)
                    return cts

                def oproj_gen(qb, cts):
                    for rb in range(4):
                        po = psP.tile([128, DOUT], f32, name="po", tag="fill")
                        for hc0 in range(0, HC, 4):
                            for hc in range(hc0, hc0 + 4):
                                nc.tensor.matmul(
                                    po[:],
                                    cts[hc][:, 128 * rb : 128 * (rb + 1)],
                                    wo_t[hc][:],
                                    start=(hc == 0),
                                    stop=(hc == HC - 1),
                                )
                            yield
                        ot = work.tile([128, DOUT], f32, name="ot", tag="ot", bufs=2)
                        nc.vector.tensor_copy(out=ot[:], in_=po[:])
                        nc.sync.dma_start(
                            out=out_d[
                                512 * qb + 128 * rb : 512 * qb + 128 * (rb + 1), :
                            ],
                            in_=ot[:],
                        )
                        yield

                def chain_gens(*gens):
                    for g in gens:
                        yield from g

                # head: Q proj for the first seq half
                for p in range(4):
                    run_gen(proj_rope_gen(wq_t, 128 * p, qT[p], 0))
                # Q proj for the second half is pumped between qb0/qb1 blocks
                qproj1 = chain_gens(
                    *[proj_rope_gen(wq_t, 128 * p, qT[p], 1) for p in range(4)]
                )
                attn_emit(0, fill=qproj1)
                attn_emit(1, fill=qproj1)
                run_gen(qproj1)  # flush: qb2 needs the full qT
                og0 = oproj_gen(0, oproj_load(0))
                attn_emit(2, fill=og0)
                run_gen(og0)
                cts1 = oproj_load(1)
                cts2 = oproj_load(2)
                og12 = chain_gens(oproj_gen(1, cts1), oproj_gen(2, cts2))
                attn_emit(3, fill=og12)
                run_gen(og12)
                run_gen(oproj_gen(3, oproj_load(3)))

            if bench_iters:
                with tc.For_i(0, bench_iters, 1, name="bench"):
                    emit_body()
            else:
                emit_body()

    nc.compile()
    return nc


def prep_inputs(x, cos, sin, wq, wk, wv, wo):
    """Shard + reformat full inputs into per-core input maps."""
    bf = ml_dtypes.bfloat16
    b, s, d = x.shape
    dout = d // 4
    cos2 = np.tile(np.ascontiguousarray(cos.T), (2, 1)).astype(bf)
    sinT = np.ascontiguousarray(sin.T)
    sinsw = np.concatenate([-sinT[:32], sinT[32:]], axis=0)
    sinsw2 = np.tile(sinsw, (2, 1)).astype(bf)
    # rotate-half permutation: out = R.T @ raw
    rotm = np.zeros((128, 128), np.float32)
    for i in range(128):
        j = (i // 64) * 64 + ((i % 64) + 32) % 64
        rotm[j, i] = 1.0
    rotm = rotm.astype(bf)

    k_loc = np.arange(128)[:, None]
    c_loc = np.arange(128)[None, :]
    mtri = (k_loc <= c_loc).astype(np.float32)
    mtri2 = np.stack([mtri, mtri], axis=1).astype(bf)  # [128, 2, 128]

    in_maps = []
    for c in range(N_CORES):
        bb, g = divmod(c, 4)
        in_maps.append(
            {
                "xT": np.ascontiguousarray(x[bb].T).astype(bf),
                "wq": np.ascontiguousarray(wq[:, 512 * g : 512 * (g + 1)]).astype(bf),
                "wk": np.ascontiguousarray(wk[:, 128 * g : 128 * (g + 1)]).astype(bf),
                "wv": np.ascontiguousarray(wv[:, 128 * g : 128 * (g + 1)]).astype(bf),
                "wo": np.ascontiguousarray(wo[:, dout * g : dout * (g + 1)]).astype(bf),
                "cos2": cos2,
                "sinsw2": sinsw2,
                "rot": rotm,
                "mtri": mtri2,
            }
        )
    return in_maps


def assemble_output(results, b, s, d):
    full = np.empty((b, s, d), np.float32)
    dout = d // 4
    for c in range(N_CORES):
        bb, g = divmod(c, 4)
        full[bb][:, dout * g : dout * (g + 1)] = results[c]["out"]
    return full


def kernel(**inputs):
    x = np.asarray(inputs["x"], np.float32)
    b, s, d = x.shape
    key = (s, d)
    if key not in _cache:
        _cache[key] = build_program(S=s, D=d)
    nc = _cache[key]
    in_maps = prep_inputs(
        x,
        np.asarray(inputs["cos"], np.float32),
        np.asarray(inputs["sin"], np.float32),
        np.asarray(inputs["wq"], np.float32),
        np.asarray(inputs["wk"], np.float32),
        np.asarray(inputs["wv"], np.float32),
        np.asarray(inputs["wo"], np.float32),
    )
    from concourse.bass_utils import run_bass_kernel_spmd

    res = run_bass_kernel_spmd(nc, in_maps, core_ids=list(range(N_CORES)))
    return assemble_output(res.results, b, s, d)


# revision 29
# speedup vs baseline: 1.0648x; 1.0016x over previous
"""Trainium2 Bass kernel for GQA attention (nn_Attention_15015205667492).

Reference computation (per batch b, seq s=2048, d=2048):
  q = (x @ wq)  -> 32 heads x 64     (RoPE)
  k = (x @ wk)  ->  8 kv heads x 64  (RoPE)
  v = (x @ wv)  ->  8 kv heads x 64
  causal softmax(q k^T / 8) @ v  (GQA: kv head = q head // 4)
  out = attn @ wo
Sharding (8 cores): DP2 x TP4.
  core c: batch = c//4, head-group g = c%4 (Q heads 8g..8g+7, KV heads 2g, 2g+1).

All matmuls bf16 (fp8 quantization error ~2.7%/operand does not average down
over random-sign dot products and blows the 2e-2 gate).  Layout:
  - x transposed + bf16 (xT [d, s]) so projections contract d on partitions.
  - Scores transposed per 128-key block (S^T = K^T.T @ Q^T); the kv head is
    duplicated across both 64-partition halves so a head pair's two QK
    matmuls land on PE row tiles (0,0)/(64,0) and can execute concurrently.
  - Softmax numerator exp() on ScalarE writes P^T (bf16) straight from the
    score psum; the AV matmul consumes P^T directly.  V carries a ones
    column so AV psum row 64 accumulates the denominator for free.
  - Causal masking: every masked region is a 128-col window of one shared
    lower-triangle [128,128]; diagonal blocks get a small DVE multiply
    (plus a memset for the fully-masked j=1 prefix) instead of full-block
    mask multiplies.  Blocks are emitted diagonal-first so the mask latency
    hides under the off-diagonal blocks' work.
  - PSUM drains run on VectorE; RoPE rotate-half is a PE permutation matmul;
    RoPE cos-multiply runs on gpsimd to offload DVE.
  - o_proj consumes the AllGathered [2048, 512] bf16 attention output and is
    emitted interleaved with late attention so PE fills ScalarE-bound gaps.
"""

import sys

sys.path.insert(0, "/opt/trn_rl_repo")

import numpy as np
import ml_dtypes

N_CORES = 8
H, KVH, HD = 32, 8, 64
RG = [[0, 1, 2, 3], [4, 5, 6, 7]]

_cache = {}


def build_program(S=2048, D=2048, enable_asserts=False, NO_CC=False, bench_iters=0):
    import concourse.mybir as mybir
    import concourse.tile as tile
    from concourse import bacc

    f32 = mybir.dt.float32
    bf16 = mybir.dt.bfloat16
    Exp = mybir.ActivationFunctionType.Exp

    DC = D // 128       # contraction chunks (16)
    QB = S // 512       # query blocks (4)
    KB = S // 128       # key blocks (16)
    DOUT = D // 4       # output column slice per core (512)
    HC = (H * HD) // 128  # o_proj contraction chunks (16)

    nc = bacc.Bacc(
        "TRN2",
        target_bir_lowering=False,
        debug=False,
        enable_asserts=enable_asserts,
        num_devices=N_CORES,
    )

    xT_d = nc.dram_tensor("xT", [D, S], bf16, kind="ExternalInput")
    wq_d = nc.dram_tensor("wq", [D, 512], bf16, kind="ExternalInput")
    wk_d = nc.dram_tensor("wk", [D, 128], bf16, kind="ExternalInput")
    wv_d = nc.dram_tensor("wv", [D, 128], bf16, kind="ExternalInput")
    wo_d = nc.dram_tensor("wo", [H * HD, DOUT], bf16, kind="ExternalInput")
    cos_d = nc.dram_tensor("cos2", [128, S], bf16, kind="ExternalInput")
    sin_d = nc.dram_tensor("sinsw2", [128, S], bf16, kind="ExternalInput")
    rot_d = nc.dram_tensor("rot", [128, 128], bf16, kind="ExternalInput")
    mtri_d = nc.dram_tensor("mtri", [128, 2, 128], bf16, kind="ExternalInput")
    out_d = nc.dram_tensor("out", [S, DOUT], f32, kind="ExternalOutput")

    with tile.TileContext(nc) as tc:
        with (
            tc.tile_pool(name="const", bufs=1) as const,
            tc.tile_pool(name="psA", bufs=2, space="PSUM") as psA,
            tc.tile_pool(name="psAV", bufs=1, space="PSUM") as psAV,
            tc.tile_pool(name="psP", bufs=2, space="PSUM") as psP,
            tc.tile_pool(name="work", bufs=2) as work,
            tc.tile_pool(name="dram", bufs=1, space="DRAM") as dram,
        ):
            # ------------- constant DMAs (issue order matters) -------------
            wk_t, wv_t = [], []
            for i in range(DC):
                t = const.tile([128, 128], bf16, name=f"wk{i}", tag=f"wk{i}")
                nc.sync.dma_start(out=t[:], in_=wk_d[128 * i : 128 * (i + 1), :])
                wk_t.append(t)
                t = const.tile([128, 128], bf16, name=f"wv{i}", tag=f"wv{i}")
                nc.sync.dma_start(out=t[:], in_=wv_d[128 * i : 128 * (i + 1), :])
                wv_t.append(t)
            rot_sb = const.tile([128, 128], bf16, name="rot", tag="rot")
            nc.sync.dma_start(out=rot_sb[:], in_=rot_d[:, :])
            mtri_sb = const.tile([128, 2, 128], bf16, name="mtri", tag="mtri")
            nc.sync.dma_start(out=mtri_sb[:], in_=mtri_d[:, :, :])
            xt = [
                const.tile([128, S], bf16, name=f"xt{i}", tag=f"xt{i}")
                for i in range(DC)
            ]
            # column-chunked, qc-major: the first seq chunk of every
            # contraction tile lands early so K/V/Q projections start ~7us in
            for qc in range(S // 512):
                for i in range(DC):
                    nc.sync.dma_start(
                        out=xt[i][:, 512 * qc : 512 * (qc + 1)],
                        in_=xT_d[128 * i : 128 * (i + 1), 512 * qc : 512 * (qc + 1)],
                    )
            cos_sb = const.tile([128, S], bf16, name="cos", tag="cos")
            nc.sync.dma_start(out=cos_sb[:], in_=cos_d[:, :])
            sin_sb = const.tile([128, S], bf16, name="sin", tag="sin")
            nc.sync.dma_start(out=sin_sb[:], in_=sin_d[:, :])
            wq_t = []
            for i in range(DC):
                t = const.tile([128, 512], bf16, name=f"wq{i}", tag=f"wq{i}")
                nc.sync.dma_start(out=t[:], in_=wq_d[128 * i : 128 * (i + 1), :])
                wq_t.append(t)
            wo_t = []
            for i in range(HC):
                t = const.tile([128, DOUT], bf16, name=f"wo{i}", tag=f"wo{i}")
                nc.sync.dma_start(out=t[:], in_=wo_d[128 * i : 128 * (i + 1), :])
                wo_t.append(t)
            ones_sb = const.tile([65, 64], bf16, name="ones", tag="ones")
            nc.vector.memset(ones_sb[:], 1.0)

            def emit_body():
                # ------------- Q/K projection + RoPE -------------
                def proj_rope_gen(w_tiles, col0, dest, c2):
                    # one 1024-col seq chunk: project + RoPE into dest.
                    # Yields between small emission units so the chunks can be
                    # interleaved ("pumped") between attention blocks.
                    raw = work.tile([128, 1024], bf16, name="raw", tag="raw", bufs=2)
                    tmp = work.tile([128, 1024], bf16, name="ropetmp", tag="ropetmp", bufs=2)
                    for q2 in range(2):
                        qc = 2 * c2 + q2
                        pq = psP.tile([128, 512], f32, name="pq", tag="fill")
                        for dc0 in range(0, DC, 4):
                            for dc in range(dc0, dc0 + 4):
                                nc.tensor.matmul(
                                    pq[:],
                                    w_tiles[dc][:, col0 : col0 + 128],
                                    xt[dc][:, 512 * qc : 512 * (qc + 1)],
                                    start=(dc == 0),
                                    stop=(dc == DC - 1),
                                )
                            yield
                        nc.vector.tensor_copy(
                            out=raw[:, 512 * q2 : 512 * (q2 + 1)], in_=pq[:]
                        )
                        yield
                    # rotate-half via PE permutation, sign folded into sinsw2
                    for q2 in range(2):
                        pr = psP.tile([128, 512], f32, name="pr", tag="fill")
                        nc.tensor.matmul(
                            pr[:],
                            rot_sb[:],
                            raw[:, 512 * q2 : 512 * (q2 + 1)],
                            start=True,
                            stop=True,
                        )
                        nc.vector.tensor_mul(
                            tmp[:, 512 * q2 : 512 * (q2 + 1)],
                            pr[:],
                            sin_sb[:, 1024 * c2 + 512 * q2 : 1024 * c2 + 512 * (q2 + 1)],
                        )
                        yield
                    nc.gpsimd.tensor_mul(
                        raw[:], raw[:], cos_sb[:, 1024 * c2 : 1024 * (c2 + 1)]
                    )
                    nc.vector.tensor_add(
                        dest[:, 1024 * c2 : 1024 * (c2 + 1)], raw[:], tmp[:]
                    )
                    yield

                def run_gen(g):
                    for _ in g:
                        pass

                qT = [
                    const.tile([128, S], bf16, name=f"qT{p}", tag=f"qT{p}")
                    for p in range(4)
                ]
                krope = work.tile([128, S], bf16, name="krope", tag="krope", bufs=1)
                for c2 in range(2):
                    run_gen(proj_rope_gen(wk_t, 0, krope, c2))
                # duplicate each kv head across both 64-partition halves
                kTd = []
                for h in range(2):
                    t = const.tile([128, S], bf16, name=f"kTd{h}", tag=f"kTd{h}")
                    nc.sync.dma_start(out=t[0:64, :], in_=krope[64 * h : 64 * h + 64, :])
                    nc.sync.dma_start(out=t[64:128, :], in_=krope[64 * h : 64 * h + 64, :])
                    kTd.append(t)

                # ------------- V projection (natural, +ones cols) ----------
                v_sb = []
                for kb in range(KB):
                    vt = const.tile([128, 132], bf16, name=f"v{kb}", tag=f"v{kb}")
                    nc.vector.memset(vt[:, 64:65], 1.0)
                    nc.vector.memset(vt[:, 129:130], 1.0)
                    pv = psP.tile([128, 512], f32, name="pv", tag="fill")
                    for dc in range(DC):
                        nc.tensor.matmul(
                            pv[:, 0:128],
                            xt[dc][:, 128 * kb : 128 * (kb + 1)],
                            wv_t[dc][:],
                            start=(dc == 0),
                            stop=(dc == DC - 1),
                        )
                    nc.vector.tensor_copy(out=vt[:, 0:64], in_=pv[:, 0:64])
                    nc.vector.tensor_copy(out=vt[:, 65:129], in_=pv[:, 64:128])
                    v_sb.append(vt)

                # ------------- attention + AllGather + o_proj -------------
                cc_in = [
                    dram.tile([512, 512], bf16, name=f"cin{qb}", tag=f"cin{qb}")
                    for qb in range(QB)
                ]
                cc_out = [
                    dram.tile([H * HD, 512], bf16, name=f"cout{qb}", tag=f"cout{qb}")
                    for qb in range(QB)
                ]

                def attn_emit(qb, fill=None):
                    def pump(n=1):
                        if fill is not None:
                            for _ in range(n):
                                if next(fill, "done") == "done":
                                    break

                    kmax = 4 * (qb + 1)
                    # diagonal blocks first: their exp+mask latency hides
                    # under the off-diagonal blocks' QK/AV work
                    order = [4 * qb, 4 * qb + 1, 4 * qb + 2, 4 * qb + 3] + list(
                        range(4 * qb)
                    )
                    for hg in range(2):      # kv head (local)
                        for p2 in range(2):  # head pair within kv group
                            pidx = 2 * hg + p2
                            pav = psAV.tile([65, 1024], f32, name="pav", tag="pav")
                            for ki, kb in enumerate(order):
                                j = kb - 4 * qb
                                vw = 512 - 128 * j if j >= 2 else 512
                                q0 = 512 * qb + (512 - vw)
                                ps = psA.tile([128, 1024], f32, name="ps", tag="ps")
                                for i in range(2):
                                    r0 = 64 * i
                                    nc.tensor.matmul(
                                        ps[:, 512 * i : 512 * i + vw],
                                        kTd[hg][r0 : r0 + 64, 128 * kb : 128 * (kb + 1)],
                                        qT[pidx][r0 : r0 + 64, q0 : q0 + vw],
                                        start=True,
                                        stop=True,
                                    )
                                pt = work.tile(
                                    [128, 1024], bf16, name="pt", tag="pt", bufs=4
                                )
                                ps3 = ps.rearrange("p (i n) -> p i n", i=2)
                                p3 = pt.rearrange("p (i n) -> p i n", i=2)
                                e0 = 128 if j == 1 else 0
                                nc.scalar.activation(
                                    out=p3[:, :, e0:vw],
                                    in_=ps3[:, :, e0:vw],
                                    func=Exp,
                                    scale=0.125,
                                )
                                # causal masking: all masked regions are
                                # 128-col windows of one shared triangle
                                if j == 0 or j == 2 or j == 3:
                                    nc.vector.tensor_mul(
                                        p3[:, :, 0:128], p3[:, :, 0:128], mtri_sb[:]
                                    )
                                elif j == 1:
                                    nc.vector.memset(p3[:, :, 0:128], 0.0)
                                    nc.vector.tensor_mul(
                                        p3[:, :, 128:256], p3[:, :, 128:256], mtri_sb[:]
                                    )
                                for i in range(2):
                                    nc.tensor.matmul(
                                        pav[:, 512 * i + 512 - vw : 512 * (i + 1)],
                                        v_sb[kb][:, 65 * hg : 65 * hg + 65],
                                        pt[:, 512 * i : 512 * i + vw],
                                        start=(ki == 0),
                                        stop=(ki == kmax - 1),
                                    )
                                pump(1)
                            # normalize: out = O^T_unnorm * (1/colsum); the
                            # denominator broadcast runs on gpsimd so PE and
                            # the psP ring stay out of the group tail
                            ou = work.tile([65, 1024], bf16, name="ou", tag="ou", bufs=2)
                            nc.vector.tensor_copy(out=ou[:], in_=pav[:])
                            rbc = work.tile([64, 1024], f32, name="rbc", tag="rbc", bufs=1)
                            for i in range(2):
                                pb = psP.tile([64, 512], f32, name=f"pb{i}", tag="fill")
                                nc.tensor.matmul(
                                    pb[:],
                                    ones_sb[64:65, :],
                                    ou[64:65, 512 * i : 512 * (i + 1)],
                                    start=True,
                                    stop=True,
                                )
                                nc.vector.reciprocal_approx_fast(
                                    out=rbc[:, 512 * i : 512 * (i + 1)], in_=pb[:]
                                )
                            at = work.tile([64, 1024], bf16, name="at", tag="at")
                            nc.vector.tensor_mul(at[:], ou[0:64, :], rbc[:])
                            pump(3)
                            for i in range(2):
                                nc.sync.dma_start(
                                    out=cc_in[qb][
                                        128 * pidx + 64 * i : 128 * pidx + 64 * (i + 1), :
                                    ],
                                    in_=at[:, 512 * i : 512 * (i + 1)],
                                )
                    if NO_CC:
                        nc.sync.dma_start(out=cc_out[qb][0:512, :], in_=cc_in[qb][:, :])
                    else:
                        nc.gpsimd.collective_compute(
                            "AllGather",
                            mybir.AluOpType.bypass,
                            replica_groups=RG,
                            ins=[cc_in[qb].opt()],
                            outs=[cc_out[qb].opt()],
                        )

                def oproj_load(qb):
                    cts = []
                    for hc in range(HC):
                        t = work.tile(
                            [128, 512], bf16, name="cct", tag="cct", bufs=HC
                        )
                        nc.sync.dma_start(
                            out=t[:], in_=cc_out[qb][128 * hc : 128 * (hc + 1), :]
                        )
                        cts.append(t)
                    return cts

                def oproj_gen(qb, cts):
                    for rb in range(4):
                        po = psP.tile([128, DOUT], f32, name="po", tag="fill")
                        for hc0 in range(0, HC, 4):
                            for hc in range(hc0, hc0 + 4):
                                nc.tensor.matmul(
                                    po[:],
                                    cts[hc][:, 128 * rb : 128 * (rb + 1)],
                                    wo_t[hc][:],
                                    start=(hc == 0),
                                    stop=(hc == HC - 1),
                                )
                            yield
                        ot = work.tile([128, DOUT], f32, name="ot", tag="ot", bufs=2)
                        nc.vector.tensor_copy(out=ot[:], in_=po[:])
                        nc.sync.dma_start(
                            out=out_d[
                                512 * qb + 128 * rb : 512 * qb + 128 * (rb + 1), :
                            ],
                            in_=ot[:],
                        )
                        yield

                def chain_gens(*gens):
                    for g in gens:
                        yield from g

                # head: Q proj for the first seq half
                for p in range(4):
                    run_gen(proj_rope_gen(wq_t, 128 * p, qT[p], 0))
                # Q proj for the second half is pumped between qb0/qb1 blocks
                qproj1 = chain_gens(
                    *[proj_rope_gen(wq_t, 128 * p, qT[p], 1) for p in range(4)]
                )
                attn_emit(0, fill=qproj1)
                attn_emit(1, fill=qproj1)
                run_gen(qproj1)  # flush: qb2 needs the full qT
                og0 = oproj_gen(0, oproj_load(0))
                attn_emit(2, fill=og0)
                run_gen(og0)
                cts1 = oproj_load(1)
                cts2 = oproj_load(2)
                og12 = chain_gens(oproj_gen(1, cts1), oproj_gen(2, cts2))
                attn_emit(3, fill=og12)
                run_gen(og12)
                run_gen(oproj_gen(3, oproj_load(3)))

            if bench_iters:
                with tc.For_i(0, bench_iters, 1, name="bench"):
                    emit_body()
            else:
                emit_body()

    nc.compile()
    return nc


def prep_inputs(x, cos, sin, wq, wk, wv, wo):
    """Shard + reformat full inputs into per-core input maps."""
    bf = ml_dtypes.bfloat16
    b, s, d = x.shape
    dout = d // 4
    cos2 = np.tile(np.ascontiguousarray(cos.T), (2, 1)).astype(bf)
    sinT = np.ascontiguousarray(sin.T)
    sinsw = np.concatenate([-sinT[:32], sinT[32:]], axis=0)
    sinsw2 = np.tile(sinsw, (2, 1)).astype(bf)
    # rotate-half permutation: out = R.T @ raw
    rotm = np.zeros((128, 128), np.float32)
    for i in range(128):
        j = (i // 64) * 64 + ((i % 64) + 32) % 64
        rotm[j, i] = 1.0
    rotm = rotm.astype(bf)

    k_loc = np.arange(128)[:, None]
    c_loc = np.arange(128)[None, :]
    mtri = (k_loc <= c_loc).astype(np.float32)
    mtri2 = np.stack([mtri, mtri], axis=1).astype(bf)  # [128, 2, 128]

    in_maps = []
    for c in range(N_CORES):
        bb, g = divmod(c, 4)
        in_maps.append(
            {
                "xT": np.ascontiguousarray(x[bb].T).astype(bf),
                "wq": np.ascontiguousarray(wq[:, 512 * g : 512 * (g + 1)]).astype(bf),
                "wk": np.ascontiguousarray(wk[:, 128 * g : 128 * (g + 1)]).astype(bf),
                "wv": np.ascontiguousarray(wv[:, 128 * g : 128 * (g + 1)]).astype(bf),
                "wo": np.ascontiguousarray(wo[:, dout * g : dout * (g + 1)]).astype(bf),
                "cos2": cos2,
                "sinsw2": sinsw2,
                "rot": rotm,
                "mtri": mtri2,
            }
        )
    return in_maps


def assemble_output(results, b, s, d):
    full = np.empty((b, s, d), np.float32)
    dout = d // 4
    for c in range(N_CORES):
        bb, g = divmod(c, 4)
        full[bb][:, dout * g : dout * (g + 1)] = results[c]["out"]
    return full


def kernel(**inputs):
    x = np.asarray(inputs["x"], np.float32)
    b, s, d = x.shape
    key = (s, d)
    if key not in _cache:
        _cache[key] = build_program(S=s, D=d)
    nc = _cache[key]
    in_maps = prep_inputs(
        x,
        np.asarray(inputs["cos"], np.float32),
        np.asarray(inputs["sin"], np.float32),
        np.asarray(inputs["wq"], np.float32),
        np.asarray(inputs["wk"], np.float32),
        np.asarray(inputs["wv"], np.float32),
        np.asarray(inputs["wo"], np.float32),
    )
    from concourse.bass_utils import run_bass_kernel_spmd

    res = run_bass_kernel_spmd(nc, in_maps, core_ids=list(range(N_CORES)))
    return assemble_output(res.results, b, s, d)


# revision 30
# speedup vs baseline: 1.0810x; 1.0152x over previous
"""Trainium2 Bass kernel for GQA attention (nn_Attention_15015205667492).

Reference computation (per batch b, seq s=2048, d=2048):
  q = (x @ wq)  -> 32 heads x 64     (RoPE)
  k = (x @ wk)  ->  8 kv heads x 64  (RoPE)
  v = (x @ wv)  ->  8 kv heads x 64
  causal softmax(q k^T / 8) @ v  (GQA: kv head = q head // 4)
  out = attn @ wo
Sharding (8 cores): DP2 x TP4.
  core c: batch = c//4, head-group g = c%4 (Q heads 8g..8g+7, KV heads 2g, 2g+1).

All matmuls bf16 (fp8 quantization error ~2.7%/operand does not average down
over random-sign dot products and blows the 2e-2 gate).  Layout:
  - x transposed + bf16 (xT [d, s]) so projections contract d on partitions.
  - Scores transposed per 128-key block (S^T = K^T.T @ Q^T); the kv head is
    duplicated across both 64-partition halves so a head pair's two QK
    matmuls land on PE row tiles (0,0)/(64,0) and can execute concurrently.
  - Softmax numerator exp() on ScalarE writes P^T (bf16) straight from the
    score psum; the AV matmul consumes P^T directly.  V carries a ones
    column so AV psum row 64 accumulates the denominator for free.
  - Causal masking: every masked region is a 128-col window of one shared
    lower-triangle [128,128]; diagonal blocks get a small DVE multiply
    (plus a memset for the fully-masked j=1 prefix) instead of full-block
    mask multiplies.  Blocks are emitted diagonal-first so the mask latency
    hides under the off-diagonal blocks' work.
  - PSUM drains run on VectorE; RoPE rotate-half is a PE permutation matmul;
    RoPE cos-multiply runs on gpsimd to offload DVE.
  - o_proj consumes the AllGathered [2048, 512] bf16 attention output and is
    emitted interleaved with late attention so PE fills ScalarE-bound gaps.
"""

import sys

sys.path.insert(0, "/opt/trn_rl_repo")

import numpy as np
import ml_dtypes

N_CORES = 8
H, KVH, HD = 32, 8, 64
RG = [[0, 1, 2, 3], [4, 5, 6, 7]]

_cache = {}


def build_program(S=2048, D=2048, enable_asserts=False, NO_CC=False, bench_iters=0):
    import concourse.mybir as mybir
    import concourse.tile as tile
    from concourse import bacc

    f32 = mybir.dt.float32
    bf16 = mybir.dt.bfloat16
    Exp = mybir.ActivationFunctionType.Exp

    DC = D // 128       # contraction chunks (16)
    QB = S // 512       # query blocks (4)
    KB = S // 128       # key blocks (16)
    DOUT = D // 4       # output column slice per core (512)
    HC = (H * HD) // 128  # o_proj contraction chunks (16)

    nc = bacc.Bacc(
        "TRN2",
        target_bir_lowering=False,
        debug=False,
        enable_asserts=enable_asserts,
        num_devices=N_CORES,
    )

    xT_d = nc.dram_tensor("xT", [D, S], bf16, kind="ExternalInput")
    wq_d = nc.dram_tensor("wq", [D, 512], bf16, kind="ExternalInput")
    wk_d = nc.dram_tensor("wk", [D, 128], bf16, kind="ExternalInput")
    wv_d = nc.dram_tensor("wv", [D, 128], bf16, kind="ExternalInput")
    wo_d = nc.dram_tensor("wo", [H * HD, DOUT], bf16, kind="ExternalInput")
    cos_d = nc.dram_tensor("cos2", [128, S], bf16, kind="ExternalInput")
    sin_d = nc.dram_tensor("sinsw2", [128, S], bf16, kind="ExternalInput")
    rot_d = nc.dram_tensor("rot", [128, 128], bf16, kind="ExternalInput")
    mtri_d = nc.dram_tensor("mtri", [128, 2, 128], bf16, kind="ExternalInput")
    out_d = nc.dram_tensor("out", [S, DOUT], f32, kind="ExternalOutput")

    with tile.TileContext(nc) as tc:
        with (
            tc.tile_pool(name="const", bufs=1) as const,
            tc.tile_pool(name="psA", bufs=2, space="PSUM") as psA,
            tc.tile_pool(name="psAV", bufs=1, space="PSUM") as psAV,
            tc.tile_pool(name="psP", bufs=2, space="PSUM") as psP,
            tc.tile_pool(name="work", bufs=2) as work,
            tc.tile_pool(name="dram", bufs=1, space="DRAM") as dram,
        ):
            # ------------- constant DMAs (issue order matters) -------------
            wk_t, wv_t = [], []
            for i in range(DC):
                t = const.tile([128, 128], bf16, name=f"wk{i}", tag=f"wk{i}")
                nc.sync.dma_start(out=t[:], in_=wk_d[128 * i : 128 * (i + 1), :])
                wk_t.append(t)
                t = const.tile([128, 128], bf16, name=f"wv{i}", tag=f"wv{i}")
                nc.sync.dma_start(out=t[:], in_=wv_d[128 * i : 128 * (i + 1), :])
                wv_t.append(t)
            rot_sb = const.tile([128, 128], bf16, name="rot", tag="rot")
            nc.sync.dma_start(out=rot_sb[:], in_=rot_d[:, :])
            mtri_sb = const.tile([128, 2, 128], bf16, name="mtri", tag="mtri")
            nc.sync.dma_start(out=mtri_sb[:], in_=mtri_d[:, :, :])
            cos_sb = const.tile([128, S], bf16, name="cos", tag="cos")
            nc.sync.dma_start(out=cos_sb[:], in_=cos_d[:, :])
            sin_sb = const.tile([128, S], bf16, name="sin", tag="sin")
            nc.sync.dma_start(out=sin_sb[:], in_=sin_d[:, :])
            xt = [
                const.tile([128, S], bf16, name=f"xt{i}", tag=f"xt{i}")
                for i in range(DC)
            ]
            # column-chunked, qc-major: the first seq chunk of every
            # contraction tile lands early so K/V/Q projections start ~7us in
            for qc in range(S // 512):
                for i in range(DC):
                    nc.sync.dma_start(
                        out=xt[i][:, 512 * qc : 512 * (qc + 1)],
                        in_=xT_d[128 * i : 128 * (i + 1), 512 * qc : 512 * (qc + 1)],
                    )
            wq_t = []
            for i in range(DC):
                t = const.tile([128, 512], bf16, name=f"wq{i}", tag=f"wq{i}")
                nc.sync.dma_start(out=t[:], in_=wq_d[128 * i : 128 * (i + 1), :])
                wq_t.append(t)
            wo_t = []
            for i in range(HC):
                t = const.tile([128, DOUT], bf16, name=f"wo{i}", tag=f"wo{i}")
                nc.sync.dma_start(out=t[:], in_=wo_d[128 * i : 128 * (i + 1), :])
                wo_t.append(t)
            ones_sb = const.tile([65, 64], bf16, name="ones", tag="ones")
            nc.vector.memset(ones_sb[:], 1.0)

            def emit_body():
                # ------------- Q/K projection + RoPE -------------
                def proj_rope_gen(w_tiles, col0, dest, c2):
                    # one 1024-col seq chunk: project + RoPE into dest.
                    # Yields between small emission units so the chunks can be
                    # interleaved ("pumped") between attention blocks.
                    raw = work.tile([128, 1024], bf16, name="raw", tag="raw", bufs=2)
                    tmp = work.tile([128, 1024], bf16, name="ropetmp", tag="ropetmp", bufs=2)
                    for q2 in range(2):
                        qc = 2 * c2 + q2
                        pq = psP.tile([128, 512], f32, name="pq", tag="fill")
                        for dc0 in range(0, DC, 4):
                            for dc in range(dc0, dc0 + 4):
                                nc.tensor.matmul(
                                    pq[:],
                                    w_tiles[dc][:, col0 : col0 + 128],
                                    xt[dc][:, 512 * qc : 512 * (qc + 1)],
                                    start=(dc == 0),
                                    stop=(dc == DC - 1),
                                )
                            yield
                        nc.vector.tensor_copy(
                            out=raw[:, 512 * q2 : 512 * (q2 + 1)], in_=pq[:]
                        )
                        yield
                    # rotate-half via PE permutation, sign folded into sinsw2
                    for q2 in range(2):
                        pr = psP.tile([128, 512], f32, name="pr", tag="fill")
                        nc.tensor.matmul(
                            pr[:],
                            rot_sb[:],
                            raw[:, 512 * q2 : 512 * (q2 + 1)],
                            start=True,
                            stop=True,
                        )
                        nc.vector.tensor_mul(
                            tmp[:, 512 * q2 : 512 * (q2 + 1)],
                            pr[:],
                            sin_sb[:, 1024 * c2 + 512 * q2 : 1024 * c2 + 512 * (q2 + 1)],
                        )
                        yield
                    nc.gpsimd.tensor_mul(
                        raw[:], raw[:], cos_sb[:, 1024 * c2 : 1024 * (c2 + 1)]
                    )
                    nc.vector.tensor_add(
                        dest[:, 1024 * c2 : 1024 * (c2 + 1)], raw[:], tmp[:]
                    )
                    yield

                def run_gen(g):
                    for _ in g:
                        pass

                qT = [
                    const.tile([128, S], bf16, name=f"qT{p}", tag=f"qT{p}")
                    for p in range(4)
                ]
                krope = work.tile([128, S], bf16, name="krope", tag="krope", bufs=1)
                for c2 in range(2):
                    run_gen(proj_rope_gen(wk_t, 0, krope, c2))
                # duplicate each kv head across both 64-partition halves
                kTd = []
                for h in range(2):
                    t = const.tile([128, S], bf16, name=f"kTd{h}", tag=f"kTd{h}")
                    nc.sync.dma_start(out=t[0:64, :], in_=krope[64 * h : 64 * h + 64, :])
                    nc.sync.dma_start(out=t[64:128, :], in_=krope[64 * h : 64 * h + 64, :])
                    kTd.append(t)

                # ------------- V projection (natural, +ones cols) ----------
                v_sb = []
                for kb in range(KB):
                    vt = const.tile([128, 132], bf16, name=f"v{kb}", tag=f"v{kb}")
                    nc.vector.memset(vt[:, 64:65], 1.0)
                    nc.vector.memset(vt[:, 129:130], 1.0)
                    pv = psP.tile([128, 512], f32, name="pv", tag="fill")
                    for dc in range(DC):
                        nc.tensor.matmul(
                            pv[:, 0:128],
                            xt[dc][:, 128 * kb : 128 * (kb + 1)],
                            wv_t[dc][:],
                            start=(dc == 0),
                            stop=(dc == DC - 1),
                        )
                    nc.vector.tensor_copy(out=vt[:, 0:64], in_=pv[:, 0:64])
                    nc.vector.tensor_copy(out=vt[:, 65:129], in_=pv[:, 64:128])
                    v_sb.append(vt)

                # ------------- attention + AllGather + o_proj -------------
                cc_in = [
                    dram.tile([512, 512], bf16, name=f"cin{qb}", tag=f"cin{qb}")
                    for qb in range(QB)
                ]
                cc_out = [
                    dram.tile([H * HD, 512], bf16, name=f"cout{qb}", tag=f"cout{qb}")
                    for qb in range(QB)
                ]

                def attn_emit(qb, fill=None):
                    def pump(n=1):
                        if fill is not None:
                            for _ in range(n):
                                if next(fill, "done") == "done":
                                    break

                    kmax = 4 * (qb + 1)
                    # diagonal blocks first: their exp+mask latency hides
                    # under the off-diagonal blocks' QK/AV work
                    order = [4 * qb, 4 * qb + 1, 4 * qb + 2, 4 * qb + 3] + list(
                        range(4 * qb)
                    )
                    for hg in range(2):      # kv head (local)
                        for p2 in range(2):  # head pair within kv group
                            pidx = 2 * hg + p2
                            pav = psAV.tile([65, 1024], f32, name="pav", tag="pav")
                            for ki, kb in enumerate(order):
                                j = kb - 4 * qb
                                vw = 512 - 128 * j if j >= 2 else 512
                                q0 = 512 * qb + (512 - vw)
                                ps = psA.tile([128, 1024], f32, name="ps", tag="ps")
                                for i in range(2):
                                    r0 = 64 * i
                                    nc.tensor.matmul(
                                        ps[:, 512 * i : 512 * i + vw],
                                        kTd[hg][r0 : r0 + 64, 128 * kb : 128 * (kb + 1)],
                                        qT[pidx][r0 : r0 + 64, q0 : q0 + vw],
                                        start=True,
                                        stop=True,
                                    )
                                pt = work.tile(
                                    [128, 1024], bf16, name="pt", tag="pt", bufs=4
                                )
                                ps3 = ps.rearrange("p (i n) -> p i n", i=2)
                                p3 = pt.rearrange("p (i n) -> p i n", i=2)
                                e0 = 128 if j == 1 else 0
                                nc.scalar.activation(
                                    out=p3[:, :, e0:vw],
                                    in_=ps3[:, :, e0:vw],
                                    func=Exp,
                                    scale=0.125,
                                )
                                # causal masking: all masked regions are
                                # 128-col windows of one shared triangle
                                if j == 0 or j == 2 or j == 3:
                                    nc.vector.tensor_mul(
                                        p3[:, :, 0:128], p3[:, :, 0:128], mtri_sb[:]
                                    )
                                elif j == 1:
                                    nc.vector.memset(p3[:, :, 0:128], 0.0)
                                    nc.vector.tensor_mul(
                                        p3[:, :, 128:256], p3[:, :, 128:256], mtri_sb[:]
                                    )
                                for i in range(2):
                                    nc.tensor.matmul(
                                        pav[:, 512 * i + 512 - vw : 512 * (i + 1)],
                                        v_sb[kb][:, 65 * hg : 65 * hg + 65],
                                        pt[:, 512 * i : 512 * i + vw],
                                        start=(ki == 0),
                                        stop=(ki == kmax - 1),
                                    )
                                pump(1)
                            # normalize: out = O^T_unnorm * (1/colsum); the
                            # denominator broadcast runs on gpsimd so PE and
                            # the psP ring stay out of the group tail
                            ou = work.tile([65, 1024], bf16, name="ou", tag="ou", bufs=2)
                            nc.vector.tensor_copy(out=ou[:], in_=pav[:])
                            rbc = work.tile([64, 1024], f32, name="rbc", tag="rbc", bufs=1)
                            for i in range(2):
                                pb = psP.tile([64, 512], f32, name=f"pb{i}", tag="fill")
                                nc.tensor.matmul(
                                    pb[:],
                                    ones_sb[64:65, :],
                                    ou[64:65, 512 * i : 512 * (i + 1)],
                                    start=True,
                                    stop=True,
                                )
                                nc.vector.reciprocal_approx_fast(
                                    out=rbc[:, 512 * i : 512 * (i + 1)], in_=pb[:]
                                )
                            at = work.tile([64, 1024], bf16, name="at", tag="at")
                            nc.vector.tensor_mul(at[:], ou[0:64, :], rbc[:])
                            pump(3)
                            for i in range(2):
                                nc.sync.dma_start(
                                    out=cc_in[qb][
                                        128 * pidx + 64 * i : 128 * pidx + 64 * (i + 1), :
                                    ],
                                    in_=at[:, 512 * i : 512 * (i + 1)],
                                )
                    if NO_CC:
                        nc.sync.dma_start(out=cc_out[qb][0:512, :], in_=cc_in[qb][:, :])
                    else:
                        nc.gpsimd.collective_compute(
                            "AllGather",
                            mybir.AluOpType.bypass,
                            replica_groups=RG,
                            ins=[cc_in[qb].opt()],
                            outs=[cc_out[qb].opt()],
                        )

                def oproj_load(qb):
                    cts = []
                    for hc in range(HC):
                        t = work.tile(
                            [128, 512], bf16, name="cct", tag="cct", bufs=HC
                        )
                        nc.sync.dma_start(
                            out=t[:], in_=cc_out[qb][128 * hc : 128 * (hc + 1), :]
                        )
                        cts.append(t)
                    return cts

                def oproj_gen(qb, cts):
                    for rb in range(4):
                        po = psP.tile([128, DOUT], f32, name="po", tag="fill")
                        for hc0 in range(0, HC, 4):
                            for hc in range(hc0, hc0 + 4):
                                nc.tensor.matmul(
                                    po[:],
                                    cts[hc][:, 128 * rb : 128 * (rb + 1)],
                                    wo_t[hc][:],
                                    start=(hc == 0),
                                    stop=(hc == HC - 1),
                                )
                            yield
                        ot = work.tile([128, DOUT], f32, name="ot", tag="ot", bufs=2)
                        nc.vector.tensor_copy(out=ot[:], in_=po[:])
                        nc.sync.dma_start(
                            out=out_d[
                                512 * qb + 128 * rb : 512 * qb + 128 * (rb + 1), :
                            ],
                            in_=ot[:],
                        )
                        yield

                def chain_gens(*gens):
                    for g in gens:
                        yield from g

                # head: Q proj for the first seq half
                for p in range(4):
                    run_gen(proj_rope_gen(wq_t, 128 * p, qT[p], 0))
                # Q proj for the second half is pumped between qb0/qb1 blocks
                qproj1 = chain_gens(
                    *[proj_rope_gen(wq_t, 128 * p, qT[p], 1) for p in range(4)]
                )
                attn_emit(0, fill=qproj1)
                attn_emit(1, fill=qproj1)
                run_gen(qproj1)  # flush: qb2 needs the full qT
                og0 = oproj_gen(0, oproj_load(0))
                attn_emit(2, fill=og0)
                run_gen(og0)
                cts1 = oproj_load(1)
                cts2 = oproj_load(2)
                og12 = chain_gens(oproj_gen(1, cts1), oproj_gen(2, cts2))
                attn_emit(3, fill=og12)
                run_gen(og12)
                run_gen(oproj_gen(3, oproj_load(3)))

            if bench_iters:
                with tc.For_i(0, bench_iters, 1, name="bench"):
                    emit_body()
            else:
                emit_body()

    nc.compile()
    return nc


def prep_inputs(x, cos, sin, wq, wk, wv, wo):
    """Shard + reformat full inputs into per-core input maps."""
    bf = ml_dtypes.bfloat16
    b, s, d = x.shape
    dout = d // 4
    cos2 = np.tile(np.ascontiguousarray(cos.T), (2, 1)).astype(bf)
    sinT = np.ascontiguousarray(sin.T)
    sinsw = np.concatenate([-sinT[:32], sinT[32:]], axis=0)
    sinsw2 = np.tile(sinsw, (2, 1)).astype(bf)
    # rotate-half permutation: out = R.T @ raw
    rotm = np.zeros((128, 128), np.float32)
    for i in range(128):
        j = (i // 64) * 64 + ((i % 64) + 32) % 64
        rotm[j, i] = 1.0
    rotm = rotm.astype(bf)

    k_loc = np.arange(128)[:, None]
    c_loc = np.arange(128)[None, :]
    mtri = (k_loc <= c_loc).astype(np.float32)
    mtri2 = np.stack([mtri, mtri], axis=1).astype(bf)  # [128, 2, 128]

    in_maps = []
    for c in range(N_CORES):
        bb, g = divmod(c, 4)
        in_maps.append(
            {
                "xT": np.ascontiguousarray(x[bb].T).astype(bf),
                "wq": np.ascontiguousarray(wq[:, 512 * g : 512 * (g + 1)]).astype(bf),
                "wk": np.ascontiguousarray(wk[:, 128 * g : 128 * (g + 1)]).astype(bf),
                "wv": np.ascontiguousarray(wv[:, 128 * g : 128 * (g + 1)]).astype(bf),
                "wo": np.ascontiguousarray(wo[:, dout * g : dout * (g + 1)]).astype(bf),
                "cos2": cos2,
                "sinsw2": sinsw2,
                "rot": rotm,
                "mtri": mtri2,
            }
        )
    return in_maps


def assemble_output(results, b, s, d):
    full = np.empty((b, s, d), np.float32)
    dout = d // 4
    for c in range(N_CORES):
        bb, g = divmod(c, 4)
        full[bb][:, dout * g : dout * (g + 1)] = results[c]["out"]
    return full


def kernel(**inputs):
    x = np.asarray(inputs["x"], np.float32)
    b, s, d = x.shape
    key = (s, d)
    if key not in _cache:
        _cache[key] = build_program(S=s, D=d)
    nc = _cache[key]
    in_maps = prep_inputs(
        x,
        np.asarray(inputs["cos"], np.float32),
        np.asarray(inputs["sin"], np.float32),
        np.asarray(inputs["wq"], np.float32),
        np.asarray(inputs["wk"], np.float32),
        np.asarray(inputs["wv"], np.float32),
        np.asarray(inputs["wo"], np.float32),
    )
    from concourse.bass_utils import run_bass_kernel_spmd

    res = run_bass_kernel_spmd(nc, in_maps, core_ids=list(range(N_CORES)))
    return assemble_output(res.results, b, s, d)


# revision 31
# speedup vs baseline: 1.1066x; 1.0237x over previous
"""Trainium2 Bass kernel for GQA attention (nn_Attention_15015205667492).

Reference computation (per batch b, seq s=2048, d=2048):
  q = (x @ wq)  -> 32 heads x 64     (RoPE)
  k = (x @ wk)  ->  8 kv heads x 64  (RoPE)
  v = (x @ wv)  ->  8 kv heads x 64
  causal softmax(q k^T / 8) @ v  (GQA: kv head = q head // 4)
  out = attn @ wo
Sharding (8 cores): DP2 x TP4.
  core c: batch = c//4, head-group g = c%4 (Q heads 8g..8g+7, KV heads 2g, 2g+1).

All matmuls bf16 (fp8 quantization error ~2.7%/operand does not average down
over random-sign dot products and blows the 2e-2 gate).  Layout:
  - x transposed + bf16 (xT [d, s]) so projections contract d on partitions.
  - Scores transposed per 128-key block (S^T = K^T.T @ Q^T); the kv head is
    duplicated across both 64-partition halves so a head pair's two QK
    matmuls land on PE row tiles (0,0)/(64,0) and can execute concurrently.
  - Softmax numerator exp() on ScalarE writes P^T (bf16) straight from the
    score psum; the AV matmul consumes P^T directly.  V carries a ones
    column so AV psum row 64 accumulates the denominator for free.
  - Causal masking: every masked region is a 128-col window of one shared
    lower-triangle [128,128]; diagonal blocks get a small DVE multiply
    (plus a memset for the fully-masked j=1 prefix) instead of full-block
    mask multiplies.  Blocks are emitted diagonal-first so the mask latency
    hides under the off-diagonal blocks' work.
  - PSUM drains run on VectorE; RoPE rotate-half is a PE permutation matmul;
    RoPE cos-multiply runs on gpsimd to offload DVE.
  - o_proj consumes the AllGathered [2048, 512] bf16 attention output and is
    emitted interleaved with late attention so PE fills ScalarE-bound gaps.
"""

import sys

sys.path.insert(0, "/opt/trn_rl_repo")

import numpy as np
import ml_dtypes

N_CORES = 8
H, KVH, HD = 32, 8, 64
RG = [[0, 1, 2, 3], [4, 5, 6, 7]]

_cache = {}


def build_program(S=2048, D=2048, enable_asserts=False, NO_CC=False, bench_iters=0):
    import concourse.mybir as mybir
    import concourse.tile as tile
    from concourse import bacc

    f32 = mybir.dt.float32
    bf16 = mybir.dt.bfloat16
    Exp = mybir.ActivationFunctionType.Exp

    DC = D // 128       # contraction chunks (16)
    QB = S // 512       # query blocks (4)
    KB = S // 128       # key blocks (16)
    DOUT = D // 4       # output column slice per core (512)
    HC = (H * HD) // 128  # o_proj contraction chunks (16)

    nc = bacc.Bacc(
        "TRN2",
        target_bir_lowering=False,
        debug=False,
        enable_asserts=enable_asserts,
        num_devices=N_CORES,
    )

    xT_d = nc.dram_tensor("xT", [D, S], bf16, kind="ExternalInput")
    wq_d = nc.dram_tensor("wq", [D, 512], bf16, kind="ExternalInput")
    wk_d = nc.dram_tensor("wk", [D, 128], bf16, kind="ExternalInput")
    wv_d = nc.dram_tensor("wv", [D, 128], bf16, kind="ExternalInput")
    wo_d = nc.dram_tensor("wo", [H * HD, DOUT], bf16, kind="ExternalInput")
    cos_d = nc.dram_tensor("cos2", [128, S], bf16, kind="ExternalInput")
    sin_d = nc.dram_tensor("sinsw2", [128, S], bf16, kind="ExternalInput")
    rot_d = nc.dram_tensor("rot", [128, 128], bf16, kind="ExternalInput")
    mtri_d = nc.dram_tensor("mtri", [128, 2, 128], bf16, kind="ExternalInput")
    out_d = nc.dram_tensor("out", [S, DOUT], f32, kind="ExternalOutput")

    with tile.TileContext(nc) as tc:
        with (
            tc.tile_pool(name="const", bufs=1) as const,
            tc.tile_pool(name="psA", bufs=2, space="PSUM") as psA,
            tc.tile_pool(name="psAV", bufs=1, space="PSUM") as psAV,
            tc.tile_pool(name="psP", bufs=2, space="PSUM") as psP,
            tc.tile_pool(name="work", bufs=2) as work,
            tc.tile_pool(name="dram", bufs=1, space="DRAM") as dram,
        ):
            # ------------- constant DMAs (issue order matters) -------------
            wk_t, wv_t = [], []
            for i in range(DC):
                t = const.tile([128, 128], bf16, name=f"wk{i}", tag=f"wk{i}")
                nc.sync.dma_start(out=t[:], in_=wk_d[128 * i : 128 * (i + 1), :])
                wk_t.append(t)
                t = const.tile([128, 128], bf16, name=f"wv{i}", tag=f"wv{i}")
                nc.sync.dma_start(out=t[:], in_=wv_d[128 * i : 128 * (i + 1), :])
                wv_t.append(t)
            rot_sb = const.tile([128, 128], bf16, name="rot", tag="rot")
            nc.sync.dma_start(out=rot_sb[:], in_=rot_d[:, :])
            mtri_sb = const.tile([128, 2, 128], bf16, name="mtri", tag="mtri")
            nc.sync.dma_start(out=mtri_sb[:], in_=mtri_d[:, :, :])
            cos_sb = const.tile([128, S], bf16, name="cos", tag="cos")
            nc.sync.dma_start(out=cos_sb[:], in_=cos_d[:, :])
            sin_sb = const.tile([128, S], bf16, name="sin", tag="sin")
            nc.sync.dma_start(out=sin_sb[:], in_=sin_d[:, :])
            xt = [
                const.tile([128, S], bf16, name=f"xt{i}", tag=f"xt{i}")
                for i in range(DC)
            ]
            # column-chunked, qc-major: the first seq chunk of every
            # contraction tile lands early so K/V/Q projections start ~7us in
            for qc in range(S // 512):
                for i in range(DC):
                    nc.sync.dma_start(
                        out=xt[i][:, 512 * qc : 512 * (qc + 1)],
                        in_=xT_d[128 * i : 128 * (i + 1), 512 * qc : 512 * (qc + 1)],
                    )
            wq_t = []
            for i in range(DC):
                t = const.tile([128, 512], bf16, name=f"wq{i}", tag=f"wq{i}")
                nc.sync.dma_start(out=t[:], in_=wq_d[128 * i : 128 * (i + 1), :])
                wq_t.append(t)
            wo_t = []
            for i in range(HC):
                t = const.tile([128, DOUT], bf16, name=f"wo{i}", tag=f"wo{i}")
                nc.sync.dma_start(out=t[:], in_=wo_d[128 * i : 128 * (i + 1), :])
                wo_t.append(t)
            ones_sb = const.tile([65, 64], bf16, name="ones", tag="ones")
            nc.vector.memset(ones_sb[:], 1.0)

            def emit_body():
                # ------------- Q/K projection + RoPE -------------
                def proj_rope_gen(w_tiles, col0, dest, c2):
                    # one 1024-col seq chunk: project + RoPE into dest.
                    # Yields between small emission units so the chunks can be
                    # interleaved ("pumped") between attention blocks.
                    raw = work.tile([128, 1024], bf16, name="raw", tag="raw", bufs=2)
                    tmp = work.tile([128, 1024], bf16, name="ropetmp", tag="ropetmp", bufs=2)
                    for q2 in range(2):
                        qc = 2 * c2 + q2
                        pq = psP.tile([128, 512], f32, name="pq", tag="fill")
                        for dc0 in range(0, DC, 4):
                            for dc in range(dc0, dc0 + 4):
                                nc.tensor.matmul(
                                    pq[:],
                                    w_tiles[dc][:, col0 : col0 + 128],
                                    xt[dc][:, 512 * qc : 512 * (qc + 1)],
                                    start=(dc == 0),
                                    stop=(dc == DC - 1),
                                )
                            yield
                        nc.vector.tensor_copy(
                            out=raw[:, 512 * q2 : 512 * (q2 + 1)], in_=pq[:]
                        )
                        yield
                    # rotate-half via PE permutation, sign folded into sinsw2
                    for q2 in range(2):
                        pr = psP.tile([128, 512], f32, name="pr", tag="fill")
                        nc.tensor.matmul(
                            pr[:],
                            rot_sb[:],
                            raw[:, 512 * q2 : 512 * (q2 + 1)],
                            start=True,
                            stop=True,
                        )
                        nc.vector.tensor_mul(
                            tmp[:, 512 * q2 : 512 * (q2 + 1)],
                            pr[:],
                            sin_sb[:, 1024 * c2 + 512 * q2 : 1024 * c2 + 512 * (q2 + 1)],
                        )
                        yield
                    nc.gpsimd.tensor_mul(
                        raw[:], raw[:], cos_sb[:, 1024 * c2 : 1024 * (c2 + 1)]
                    )
                    nc.vector.tensor_add(
                        dest[:, 1024 * c2 : 1024 * (c2 + 1)], raw[:], tmp[:]
                    )
                    yield

                def run_gen(g):
                    for _ in g:
                        pass

                qT = [
                    const.tile([128, S], bf16, name=f"qT{p}", tag=f"qT{p}")
                    for p in range(4)
                ]
                krope = work.tile([128, S], bf16, name="krope", tag="krope", bufs=1)
                for c2 in range(2):
                    run_gen(proj_rope_gen(wk_t, 0, krope, c2))
                # duplicate each kv head across both 64-partition halves
                kTd = []
                for h in range(2):
                    t = const.tile([128, S], bf16, name=f"kTd{h}", tag=f"kTd{h}")
                    nc.sync.dma_start(out=t[0:64, :], in_=krope[64 * h : 64 * h + 64, :])
                    nc.sync.dma_start(out=t[64:128, :], in_=krope[64 * h : 64 * h + 64, :])
                    kTd.append(t)

                # ------------- V projection -------------
                # computed transposed (wv stationary, wide N=512 moving) to
                # quarter the matmul count, then XBAR DMA-transposed back to
                # natural [keys, hd] with the ones columns for the AV psum
                vT = work.tile([128, S], bf16, name="vT", tag="vT", bufs=1)
                for sc in range(S // 512):
                    pv = psP.tile([128, 512], f32, name="pv", tag="fill")
                    for dc in range(DC):
                        nc.tensor.matmul(
                            pv[:],
                            wv_t[dc][:],
                            xt[dc][:, 512 * sc : 512 * (sc + 1)],
                            start=(dc == 0),
                            stop=(dc == DC - 1),
                        )
                    nc.vector.tensor_copy(
                        out=vT[:, 512 * sc : 512 * (sc + 1)], in_=pv[:]
                    )
                v_sb = []
                for kb in range(KB):
                    vt = const.tile([128, 132], bf16, name=f"v{kb}", tag=f"v{kb}")
                    nc.vector.memset(vt[:, 64:65], 1.0)
                    nc.vector.memset(vt[:, 129:130], 1.0)
                    for h in range(2):
                        nc.sync.dma_start_transpose(
                            out=vt[:, 65 * h : 65 * h + 64],
                            in_=vT[64 * h : 64 * h + 64, 128 * kb : 128 * (kb + 1)],
                        )
                    v_sb.append(vt)

                # ------------- attention + AllGather + o_proj -------------
                cc_in = [
                    dram.tile([512, 512], bf16, name=f"cin{qb}", tag=f"cin{qb}")
                    for qb in range(QB)
                ]
                cc_out = [
                    dram.tile([H * HD, 512], bf16, name=f"cout{qb}", tag=f"cout{qb}")
                    for qb in range(QB)
                ]

                def attn_emit(qb, fill=None):
                    def pump(n=1):
                        if fill is not None:
                            for _ in range(n):
                                if next(fill, "done") == "done":
                                    break

                    kmax = 4 * (qb + 1)
                    # diagonal blocks first: their exp+mask latency hides
                    # under the off-diagonal blocks' QK/AV work
                    order = [4 * qb, 4 * qb + 1, 4 * qb + 2, 4 * qb + 3] + list(
                        range(4 * qb)
                    )
                    for hg in range(2):      # kv head (local)
                        for p2 in range(2):  # head pair within kv group
                            pidx = 2 * hg + p2
                            pav = psAV.tile([65, 1024], f32, name="pav", tag="pav")
                            for ki, kb in enumerate(order):
                                j = kb - 4 * qb
                                vw = 512 - 128 * j if j >= 2 else 512
                                q0 = 512 * qb + (512 - vw)
                                ps = psA.tile([128, 1024], f32, name="ps", tag="ps")
                                for i in range(2):
                                    r0 = 64 * i
                                    nc.tensor.matmul(
                                        ps[:, 512 * i : 512 * i + vw],
                                        kTd[hg][r0 : r0 + 64, 128 * kb : 128 * (kb + 1)],
                                        qT[pidx][r0 : r0 + 64, q0 : q0 + vw],
                                        start=True,
                                        stop=True,
                                    )
                                pt = work.tile(
                                    [128, 1024], bf16, name="pt", tag="pt", bufs=4
                                )
                                ps3 = ps.rearrange("p (i n) -> p i n", i=2)
                                p3 = pt.rearrange("p (i n) -> p i n", i=2)
                                e0 = 128 if j == 1 else 0
                                nc.scalar.activation(
                                    out=p3[:, :, e0:vw],
                                    in_=ps3[:, :, e0:vw],
                                    func=Exp,
                                    scale=0.125,
                                )
                                # causal masking: all masked regions are
                                # 128-col windows of one shared triangle
                                if j == 0 or j == 2 or j == 3:
                                    nc.vector.tensor_mul(
                                        p3[:, :, 0:128], p3[:, :, 0:128], mtri_sb[:]
                                    )
                                elif j == 1:
                                    nc.vector.memset(p3[:, :, 0:128], 0.0)
                                    nc.vector.tensor_mul(
                                        p3[:, :, 128:256], p3[:, :, 128:256], mtri_sb[:]
                                    )
                                for i in range(2):
                                    nc.tensor.matmul(
                                        pav[:, 512 * i + 512 - vw : 512 * (i + 1)],
                                        v_sb[kb][:, 65 * hg : 65 * hg + 65],
                                        pt[:, 512 * i : 512 * i + vw],
                                        start=(ki == 0),
                                        stop=(ki == kmax - 1),
                                    )
                                pump(1)
                            # normalize: out = O^T_unnorm * (1/colsum); the
                            # denominator broadcast runs on gpsimd so PE and
                            # the psP ring stay out of the group tail
                            ou = work.tile([65, 1024], bf16, name="ou", tag="ou", bufs=2)
                            nc.vector.tensor_copy(out=ou[:], in_=pav[:])
                            rbc = work.tile([64, 1024], f32, name="rbc", tag="rbc", bufs=1)
                            for i in range(2):
                                pb = psP.tile([64, 512], f32, name=f"pb{i}", tag="fill")
                                nc.tensor.matmul(
                                    pb[:],
                                    ones_sb[64:65, :],
                                    ou[64:65, 512 * i : 512 * (i + 1)],
                                    start=True,
                                    stop=True,
                                )
                                nc.vector.reciprocal_approx_fast(
                                    out=rbc[:, 512 * i : 512 * (i + 1)], in_=pb[:]
                                )
                            at = work.tile([64, 1024], bf16, name="at", tag="at")
                            nc.vector.tensor_mul(at[:], ou[0:64, :], rbc[:])
                            pump(3)
                            for i in range(2):
                                nc.sync.dma_start(
                                    out=cc_in[qb][
                                        128 * pidx + 64 * i : 128 * pidx + 64 * (i + 1), :
                                    ],
                                    in_=at[:, 512 * i : 512 * (i + 1)],
                                )
                            if NO_CC:
                                nc.sync.dma_start(
                                    out=cc_out[qb][128 * pidx : 128 * (pidx + 1), :],
                                    in_=cc_in[qb][128 * pidx : 128 * (pidx + 1), :],
                                )
                    if NO_CC:
                        pass  # per-pidx copies emitted in the group loop above
                    else:
                        nc.gpsimd.collective_compute(
                            "AllGather",
                            mybir.AluOpType.bypass,
                            replica_groups=RG,
                            ins=[cc_in[qb].opt()],
                            outs=[cc_out[qb].opt()],
                        )

                HORDER = [4 * c + p for p in range(4) for c in range(4)]

                def oproj_load(qb):
                    cts = {}
                    for hc in HORDER:
                        t = work.tile(
                            [128, 512], bf16, name="cct", tag="cct", bufs=HC
                        )
                        nc.sync.dma_start(
                            out=t[:], in_=cc_out[qb][128 * hc : 128 * (hc + 1), :]
                        )
                        cts[hc] = t
                    return cts

                def oproj_gen(qb, cts):
                    for rb in range(4):
                        po = psP.tile([128, DOUT], f32, name="po", tag="fill")
                        for u0 in range(0, HC, 4):
                            for u in range(u0, u0 + 4):
                                hc = HORDER[u]
                                nc.tensor.matmul(
                                    po[:],
                                    cts[hc][:, 128 * rb : 128 * (rb + 1)],
                                    wo_t[hc][:],
                                    start=(u == 0),
                                    stop=(u == HC - 1),
                                )
                            yield
                        ot = work.tile([128, DOUT], f32, name="ot", tag="ot", bufs=2)
                        nc.vector.tensor_copy(out=ot[:], in_=po[:])
                        nc.sync.dma_start(
                            out=out_d[
                                512 * qb + 128 * rb : 512 * qb + 128 * (rb + 1), :
                            ],
                            in_=ot[:],
                        )
                        yield

                def chain_gens(*gens):
                    for g in gens:
                        yield from g

                # head: Q proj for the first seq half
                for p in range(4):
                    run_gen(proj_rope_gen(wq_t, 128 * p, qT[p], 0))
                # Q proj for the second half is pumped between qb0/qb1 blocks
                qproj1 = chain_gens(
                    *[proj_rope_gen(wq_t, 128 * p, qT[p], 1) for p in range(4)]
                )
                attn_emit(0, fill=qproj1)
                attn_emit(1, fill=qproj1)
                run_gen(qproj1)  # flush: qb2 needs the full qT
                og0 = oproj_gen(0, oproj_load(0))
                attn_emit(2, fill=og0)
                run_gen(og0)
                cts1 = oproj_load(1)
                cts2 = oproj_load(2)
                og12 = chain_gens(oproj_gen(1, cts1), oproj_gen(2, cts2))
                attn_emit(3, fill=og12)
                run_gen(og12)
                run_gen(oproj_gen(3, oproj_load(3)))

            if bench_iters:
                with tc.For_i(0, bench_iters, 1, name="bench"):
                    emit_body()
            else:
                emit_body()

    nc.compile()
    return nc


def prep_inputs(x, cos, sin, wq, wk, wv, wo):
    """Shard + reformat full inputs into per-core input maps."""
    bf = ml_dtypes.bfloat16
    b, s, d = x.shape
    dout = d // 4
    cos2 = np.tile(np.ascontiguousarray(cos.T), (2, 1)).astype(bf)
    sinT = np.ascontiguousarray(sin.T)
    sinsw = np.concatenate([-sinT[:32], sinT[32:]], axis=0)
    sinsw2 = np.tile(sinsw, (2, 1)).astype(bf)
    # rotate-half permutation: out = R.T @ raw
    rotm = np.zeros((128, 128), np.float32)
    for i in range(128):
        j = (i // 64) * 64 + ((i % 64) + 32) % 64
        rotm[j, i] = 1.0
    rotm = rotm.astype(bf)

    k_loc = np.arange(128)[:, None]
    c_loc = np.arange(128)[None, :]
    mtri = (k_loc <= c_loc).astype(np.float32)
    mtri2 = np.stack([mtri, mtri], axis=1).astype(bf)  # [128, 2, 128]

    in_maps = []
    for c in range(N_CORES):
        bb, g = divmod(c, 4)
        in_maps.append(
            {
                "xT": np.ascontiguousarray(x[bb].T).astype(bf),
                "wq": np.ascontiguousarray(wq[:, 512 * g : 512 * (g + 1)]).astype(bf),
                "wk": np.ascontiguousarray(wk[:, 128 * g : 128 * (g + 1)]).astype(bf),
                "wv": np.ascontiguousarray(wv[:, 128 * g : 128 * (g + 1)]).astype(bf),
                "wo": np.ascontiguousarray(wo[:, dout * g : dout * (g + 1)]).astype(bf),
                "cos2": cos2,
                "sinsw2": sinsw2,
                "rot": rotm,
                "mtri": mtri2,
            }
        )
    return in_maps


def assemble_output(results, b, s, d):
    full = np.empty((b, s, d), np.float32)
    dout = d // 4
    for c in range(N_CORES):
        bb, g = divmod(c, 4)
        full[bb][:, dout * g : dout * (g + 1)] = results[c]["out"]
    return full


def kernel(**inputs):
    x = np.asarray(inputs["x"], np.float32)
    b, s, d = x.shape
    key = (s, d)
    if key not in _cache:
        _cache[key] = build_program(S=s, D=d)
    nc = _cache[key]
    in_maps = prep_inputs(
        x,
        np.asarray(inputs["cos"], np.float32),
        np.asarray(inputs["sin"], np.float32),
        np.asarray(inputs["wq"], np.float32),
        np.asarray(inputs["wk"], np.float32),
        np.asarray(inputs["wv"], np.float32),
        np.asarray(inputs["wo"], np.float32),
    )
    from concourse.bass_utils import run_bass_kernel_spmd

    res = run_bass_kernel_spmd(nc, in_maps, core_ids=list(range(N_CORES)))
    return assemble_output(res.results, b, s, d)


# revision 32
# speedup vs baseline: 1.1124x; 1.0052x over previous
"""Trainium2 Bass kernel for GQA attention (nn_Attention_15015205667492).

Reference computation (per batch b, seq s=2048, d=2048):
  q = (x @ wq)  -> 32 heads x 64     (RoPE)
  k = (x @ wk)  ->  8 kv heads x 64  (RoPE)
  v = (x @ wv)  ->  8 kv heads x 64
  causal softmax(q k^T / 8) @ v  (GQA: kv head = q head // 4)
  out = attn @ wo
Sharding (8 cores): DP2 x TP4.
  core c: batch = c//4, head-group g = c%4 (Q heads 8g..8g+7, KV heads 2g, 2g+1).

All matmuls bf16 (fp8 quantization error ~2.7%/operand does not average down
over random-sign dot products and blows the 2e-2 gate).  Layout:
  - x transposed + bf16 (xT [d, s]) so projections contract d on partitions.
  - Scores transposed per 128-key block (S^T = K^T.T @ Q^T); the kv head is
    duplicated across both 64-partition halves so a head pair's two QK
    matmuls land on PE row tiles (0,0)/(64,0) and can execute concurrently.
  - Softmax numerator exp() on ScalarE writes P^T (bf16) straight from the
    score psum; the AV matmul consumes P^T directly.  V carries a ones
    column so AV psum row 64 accumulates the denominator for free.
  - Causal masking: every masked region is a 128-col window of one shared
    lower-triangle [128,128]; diagonal blocks get a small DVE multiply
    (plus a memset for the fully-masked j=1 prefix) instead of full-block
    mask multiplies.  Blocks are emitted diagonal-first so the mask latency
    hides under the off-diagonal blocks' work.
  - PSUM drains run on VectorE; RoPE rotate-half is a PE permutation matmul;
    RoPE cos-multiply runs on gpsimd to offload DVE.
  - o_proj consumes the AllGathered [2048, 512] bf16 attention output and is
    emitted interleaved with late attention so PE fills ScalarE-bound gaps.
"""

import sys

sys.path.insert(0, "/opt/trn_rl_repo")

import numpy as np
import ml_dtypes

N_CORES = 8
H, KVH, HD = 32, 8, 64
RG = [[0, 1, 2, 3], [4, 5, 6, 7]]

_cache = {}


def build_program(S=2048, D=2048, enable_asserts=False, NO_CC=False, bench_iters=0):
    import concourse.mybir as mybir
    import concourse.tile as tile
    from concourse import bacc

    f32 = mybir.dt.float32
    bf16 = mybir.dt.bfloat16
    Exp = mybir.ActivationFunctionType.Exp

    DC = D // 128       # contraction chunks (16)
    QB = S // 512       # query blocks (4)
    KB = S // 128       # key blocks (16)
    DOUT = D // 4       # output column slice per core (512)
    HC = (H * HD) // 128  # o_proj contraction chunks (16)

    nc = bacc.Bacc(
        "TRN2",
        target_bir_lowering=False,
        debug=False,
        enable_asserts=enable_asserts,
        num_devices=N_CORES,
    )

    xT_d = nc.dram_tensor("xT", [D, S], bf16, kind="ExternalInput")
    wq_d = nc.dram_tensor("wq", [D, 512], bf16, kind="ExternalInput")
    wk_d = nc.dram_tensor("wk", [D, 128], bf16, kind="ExternalInput")
    wv_d = nc.dram_tensor("wv", [D, 128], bf16, kind="ExternalInput")
    wo_d = nc.dram_tensor("wo", [H * HD, DOUT], bf16, kind="ExternalInput")
    cos_d = nc.dram_tensor("cos2", [128, S], bf16, kind="ExternalInput")
    sin_d = nc.dram_tensor("sinsw2", [128, S], bf16, kind="ExternalInput")
    rot_d = nc.dram_tensor("rot", [128, 128], bf16, kind="ExternalInput")
    mtri_d = nc.dram_tensor("mtri", [128, 2, 128], bf16, kind="ExternalInput")
    out_d = nc.dram_tensor("out", [S, DOUT], f32, kind="ExternalOutput")

    with tile.TileContext(nc) as tc:
        with (
            tc.tile_pool(name="const", bufs=1) as const,
            tc.tile_pool(name="psA", bufs=2, space="PSUM") as psA,
            tc.tile_pool(name="psAV", bufs=1, space="PSUM") as psAV,
            tc.tile_pool(name="psP", bufs=2, space="PSUM") as psP,
            tc.tile_pool(name="work", bufs=2) as work,
            tc.tile_pool(name="dram", bufs=1, space="DRAM") as dram,
        ):
            # ------------- constant DMAs (issue order matters) -------------
            wk_t, wv_t = [], []
            for i in range(DC):
                t = const.tile([128, 128], bf16, name=f"wk{i}", tag=f"wk{i}")
                nc.sync.dma_start(out=t[:], in_=wk_d[128 * i : 128 * (i + 1), :])
                wk_t.append(t)
                t = const.tile([128, 128], bf16, name=f"wv{i}", tag=f"wv{i}")
                nc.sync.dma_start(out=t[:], in_=wv_d[128 * i : 128 * (i + 1), :])
                wv_t.append(t)
            rot_sb = const.tile([128, 128], bf16, name="rot", tag="rot")
            nc.sync.dma_start(out=rot_sb[:], in_=rot_d[:, :])
            mtri_sb = const.tile([128, 2, 128], bf16, name="mtri", tag="mtri")
            nc.sync.dma_start(out=mtri_sb[:], in_=mtri_d[:, :, :])
            cos_sb = const.tile([128, S], bf16, name="cos", tag="cos")
            nc.sync.dma_start(out=cos_sb[:], in_=cos_d[:, :])
            sin_sb = const.tile([128, S], bf16, name="sin", tag="sin")
            nc.sync.dma_start(out=sin_sb[:], in_=sin_d[:, :])
            xt = [
                const.tile([128, S], bf16, name=f"xt{i}", tag=f"xt{i}")
                for i in range(DC)
            ]
            # column-chunked, qc-major: the first seq chunk of every
            # contraction tile lands early so K/V/Q projections start ~7us in
            for qc in range(S // 512):
                for i in range(DC):
                    nc.sync.dma_start(
                        out=xt[i][:, 512 * qc : 512 * (qc + 1)],
                        in_=xT_d[128 * i : 128 * (i + 1), 512 * qc : 512 * (qc + 1)],
                    )
            wq_t = []
            for i in range(DC):
                t = const.tile([128, 512], bf16, name=f"wq{i}", tag=f"wq{i}")
                nc.sync.dma_start(out=t[:], in_=wq_d[128 * i : 128 * (i + 1), :])
                wq_t.append(t)
            wo_t = []
            for i in range(HC):
                t = const.tile([128, DOUT], bf16, name=f"wo{i}", tag=f"wo{i}")
                nc.sync.dma_start(out=t[:], in_=wo_d[128 * i : 128 * (i + 1), :])
                wo_t.append(t)
            ones_sb = const.tile([65, 64], bf16, name="ones", tag="ones")
            nc.vector.memset(ones_sb[:], 1.0)

            def emit_body():
                # ------------- Q/K projection + RoPE -------------
                def proj_rope_gen(w_tiles, col0, dest, c2):
                    # one 1024-col seq chunk: project + RoPE into dest.
                    # Yields between small emission units so the chunks can be
                    # interleaved ("pumped") between attention blocks.
                    raw = work.tile([128, 1024], bf16, name="raw", tag="raw", bufs=2)
                    tmp = work.tile([128, 1024], bf16, name="ropetmp", tag="ropetmp", bufs=2)
                    for q2 in range(2):
                        qc = 2 * c2 + q2
                        pq = psP.tile([128, 512], f32, name="pq", tag="fill")
                        for dc0 in range(0, DC, 4):
                            for dc in range(dc0, dc0 + 4):
                                nc.tensor.matmul(
                                    pq[:],
                                    w_tiles[dc][:, col0 : col0 + 128],
                                    xt[dc][:, 512 * qc : 512 * (qc + 1)],
                                    start=(dc == 0),
                                    stop=(dc == DC - 1),
                                )
                            yield
                        nc.vector.tensor_copy(
                            out=raw[:, 512 * q2 : 512 * (q2 + 1)], in_=pq[:]
                        )
                        yield
                    # rotate-half via PE permutation, sign folded into sinsw2
                    for q2 in range(2):
                        pr = psP.tile([128, 512], f32, name="pr", tag="fill")
                        nc.tensor.matmul(
                            pr[:],
                            rot_sb[:],
                            raw[:, 512 * q2 : 512 * (q2 + 1)],
                            start=True,
                            stop=True,
                        )
                        nc.vector.tensor_mul(
                            tmp[:, 512 * q2 : 512 * (q2 + 1)],
                            pr[:],
                            sin_sb[:, 1024 * c2 + 512 * q2 : 1024 * c2 + 512 * (q2 + 1)],
                        )
                        yield
                    nc.gpsimd.tensor_mul(
                        raw[:], raw[:], cos_sb[:, 1024 * c2 : 1024 * (c2 + 1)]
                    )
                    nc.vector.tensor_add(
                        dest[:, 1024 * c2 : 1024 * (c2 + 1)], raw[:], tmp[:]
                    )
                    yield

                def run_gen(g):
                    for _ in g:
                        pass

                qT = [
                    const.tile([128, S], bf16, name=f"qT{p}", tag=f"qT{p}")
                    for p in range(4)
                ]
                krope = work.tile([128, S], bf16, name="krope", tag="krope", bufs=1)
                for c2 in range(2):
                    run_gen(proj_rope_gen(wk_t, 0, krope, c2))
                # duplicate each kv head across both 64-partition halves
                kTd = []
                for h in range(2):
                    t = const.tile([128, S], bf16, name=f"kTd{h}", tag=f"kTd{h}")
                    nc.sync.dma_start(out=t[0:64, :], in_=krope[64 * h : 64 * h + 64, :])
                    nc.sync.dma_start(out=t[64:128, :], in_=krope[64 * h : 64 * h + 64, :])
                    kTd.append(t)

                # ------------- V projection (natural, +ones cols) ----------
                v_sb = []
                for kb in range(KB):
                    vt = const.tile([128, 132], bf16, name=f"v{kb}", tag=f"v{kb}")
                    nc.vector.memset(vt[:, 64:65], 1.0)
                    nc.vector.memset(vt[:, 129:130], 1.0)
                    pv = psP.tile([128, 512], f32, name="pv", tag="fill")
                    for dc in range(DC):
                        nc.tensor.matmul(
                            pv[:, 0:128],
                            xt[dc][:, 128 * kb : 128 * (kb + 1)],
                            wv_t[dc][:],
                            start=(dc == 0),
                            stop=(dc == DC - 1),
                        )
                    nc.vector.tensor_copy(out=vt[:, 0:64], in_=pv[:, 0:64])
                    nc.vector.tensor_copy(out=vt[:, 65:129], in_=pv[:, 64:128])
                    v_sb.append(vt)

                # ------------- attention + AllGather + o_proj -------------
                cc_in = [
                    dram.tile([512, 512], bf16, name=f"cin{qb}", tag=f"cin{qb}")
                    for qb in range(QB)
                ]
                cc_out = [
                    dram.tile([H * HD, 512], bf16, name=f"cout{qb}", tag=f"cout{qb}")
                    for qb in range(QB)
                ]

                def attn_emit(qb, fill=None):
                    def pump(n=1):
                        if fill is not None:
                            for _ in range(n):
                                if next(fill, "done") == "done":
                                    break

                    kmax = 4 * (qb + 1)
                    # diagonal blocks first: their exp+mask latency hides
                    # under the off-diagonal blocks' QK/AV work
                    order = [4 * qb, 4 * qb + 1, 4 * qb + 2, 4 * qb + 3] + list(
                        range(4 * qb)
                    )
                    for hg in range(2):      # kv head (local)
                        for p2 in range(2):  # head pair within kv group
                            pidx = 2 * hg + p2
                            pav = psAV.tile([65, 1024], f32, name="pav", tag="pav")
                            for ki, kb in enumerate(order):
                                j = kb - 4 * qb
                                vw = 512 - 128 * j if j >= 2 else 512
                                q0 = 512 * qb + (512 - vw)
                                ps = psA.tile([128, 1024], f32, name="ps", tag="ps")
                                for i in range(2):
                                    r0 = 64 * i
                                    nc.tensor.matmul(
                                        ps[:, 512 * i : 512 * i + vw],
                                        kTd[hg][r0 : r0 + 64, 128 * kb : 128 * (kb + 1)],
                                        qT[pidx][r0 : r0 + 64, q0 : q0 + vw],
                                        start=True,
                                        stop=True,
                                    )
                                pt = work.tile(
                                    [128, 1024], bf16, name="pt", tag="pt", bufs=4
                                )
                                ps3 = ps.rearrange("p (i n) -> p i n", i=2)
                                p3 = pt.rearrange("p (i n) -> p i n", i=2)
                                e0 = 128 if j == 1 else 0
                                nc.scalar.activation(
                                    out=p3[:, :, e0:vw],
                                    in_=ps3[:, :, e0:vw],
                                    func=Exp,
                                    scale=0.125,
                                )
                                # causal masking: all masked regions are
                                # 128-col windows of one shared triangle
                                if j == 0 or j == 2 or j == 3:
                                    nc.vector.tensor_mul(
                                        p3[:, :, 0:128], p3[:, :, 0:128], mtri_sb[:]
                                    )
                                elif j == 1:
                                    nc.vector.memset(p3[:, :, 0:128], 0.0)
                                    nc.vector.tensor_mul(
                                        p3[:, :, 128:256], p3[:, :, 128:256], mtri_sb[:]
                                    )
                                for i in range(2):
                                    nc.tensor.matmul(
                                        pav[:, 512 * i + 512 - vw : 512 * (i + 1)],
                                        v_sb[kb][:, 65 * hg : 65 * hg + 65],
                                        pt[:, 512 * i : 512 * i + vw],
                                        start=(ki == 0),
                                        stop=(ki == kmax - 1),
                                    )
                                pump(1)
                            # normalize: out = O^T_unnorm * (1/colsum); the
                            # denominator broadcast runs on gpsimd so PE and
                            # the psP ring stay out of the group tail
                            ou = work.tile([65, 1024], bf16, name="ou", tag="ou", bufs=2)
                            nc.vector.tensor_copy(out=ou[:], in_=pav[:])
                            rbc = work.tile([64, 1024], f32, name="rbc", tag="rbc", bufs=1)
                            for i in range(2):
                                pb = psP.tile([64, 512], f32, name=f"pb{i}", tag="fill")
                                nc.tensor.matmul(
                                    pb[:],
                                    ones_sb[64:65, :],
                                    ou[64:65, 512 * i : 512 * (i + 1)],
                                    start=True,
                                    stop=True,
                                )
                                nc.vector.reciprocal_approx_fast(
                                    out=rbc[:, 512 * i : 512 * (i + 1)], in_=pb[:]
                                )
                            at = work.tile([64, 1024], bf16, name="at", tag="at")
                            nc.vector.tensor_mul(at[:], ou[0:64, :], rbc[:])
                            pump(3)
                            for i in range(2):
                                nc.sync.dma_start(
                                    out=cc_in[qb][
                                        128 * pidx + 64 * i : 128 * pidx + 64 * (i + 1), :
                                    ],
                                    in_=at[:, 512 * i : 512 * (i + 1)],
                                )
                            if NO_CC:
                                nc.sync.dma_start(
                                    out=cc_out[qb][128 * pidx : 128 * (pidx + 1), :],
                                    in_=cc_in[qb][128 * pidx : 128 * (pidx + 1), :],
                                )
                    if NO_CC:
                        pass  # per-pidx copies emitted in the group loop above
                    else:
                        nc.gpsimd.collective_compute(
                            "AllGather",
                            mybir.AluOpType.bypass,
                            replica_groups=RG,
                            ins=[cc_in[qb].opt()],
                            outs=[cc_out[qb].opt()],
                        )

                HORDER = [4 * c + p for p in range(4) for c in range(4)]

                def oproj_load(qb):
                    cts = {}
                    for hc in HORDER:
                        t = work.tile(
                            [128, 512], bf16, name="cct", tag="cct", bufs=HC
                        )
                        nc.sync.dma_start(
                            out=t[:], in_=cc_out[qb][128 * hc : 128 * (hc + 1), :]
                        )
                        cts[hc] = t
                    return cts

                def oproj_gen(qb, cts):
                    for rb in range(4):
                        po = psP.tile([128, DOUT], f32, name="po", tag="fill")
                        for u0 in range(0, HC, 4):
                            for u in range(u0, u0 + 4):
                                hc = HORDER[u]
                                nc.tensor.matmul(
                                    po[:],
                                    cts[hc][:, 128 * rb : 128 * (rb + 1)],
                                    wo_t[hc][:],
                                    start=(u == 0),
                                    stop=(u == HC - 1),
                                )
                            yield
                        ot = work.tile([128, DOUT], f32, name="ot", tag="ot", bufs=2)
                        nc.vector.tensor_copy(out=ot[:], in_=po[:])
                        nc.sync.dma_start(
                            out=out_d[
                                512 * qb + 128 * rb : 512 * qb + 128 * (rb + 1), :
                            ],
                            in_=ot[:],
                        )
                        yield

                def chain_gens(*gens):
                    for g in gens:
                        yield from g

                # head: Q proj for the first seq half
                for p in range(4):
                    run_gen(proj_rope_gen(wq_t, 128 * p, qT[p], 0))
                # Q proj for the second half is pumped between qb0/qb1 blocks
                qproj1 = chain_gens(
                    *[proj_rope_gen(wq_t, 128 * p, qT[p], 1) for p in range(4)]
                )
                attn_emit(0, fill=qproj1)
                attn_emit(1, fill=qproj1)
                run_gen(qproj1)  # flush: qb2 needs the full qT
                og0 = oproj_gen(0, oproj_load(0))
                attn_emit(2, fill=og0)
                run_gen(og0)
                cts1 = oproj_load(1)
                cts2 = oproj_load(2)
                og12 = chain_gens(oproj_gen(1, cts1), oproj_gen(2, cts2))
                attn_emit(3, fill=og12)
                run_gen(og12)
                run_gen(oproj_gen(3, oproj_load(3)))

            if bench_iters:
                with tc.For_i(0, bench_iters, 1, name="bench"):
                    emit_body()
            else:
                emit_body()

    nc.compile()
    return nc


def prep_inputs(x, cos, sin, wq, wk, wv, wo):
    """Shard + reformat full inputs into per-core input maps."""
    bf = ml_dtypes.bfloat16
    b, s, d = x.shape
    dout = d // 4
    cos2 = np.tile(np.ascontiguousarray(cos.T), (2, 1)).astype(bf)
    sinT = np.ascontiguousarray(sin.T)
    sinsw = np.concatenate([-sinT[:32], sinT[32:]], axis=0)
    sinsw2 = np.tile(sinsw, (2, 1)).astype(bf)
    # rotate-half permutation: out = R.T @ raw
    rotm = np.zeros((128, 128), np.float32)
    for i in range(128):
        j = (i // 64) * 64 + ((i % 64) + 32) % 64
        rotm[j, i] = 1.0
    rotm = rotm.astype(bf)

    k_loc = np.arange(128)[:, None]
    c_loc = np.arange(128)[None, :]
    mtri = (k_loc <= c_loc).astype(np.float32)
    mtri2 = np.stack([mtri, mtri], axis=1).astype(bf)  # [128, 2, 128]

    in_maps = []
    for c in range(N_CORES):
        bb, g = divmod(c, 4)
        in_maps.append(
            {
                "xT": np.ascontiguousarray(x[bb].T).astype(bf),
                "wq": np.ascontiguousarray(wq[:, 512 * g : 512 * (g + 1)]).astype(bf),
                "wk": np.ascontiguousarray(wk[:, 128 * g : 128 * (g + 1)]).astype(bf),
                "wv": np.ascontiguousarray(wv[:, 128 * g : 128 * (g + 1)]).astype(bf),
                "wo": np.ascontiguousarray(wo[:, dout * g : dout * (g + 1)]).astype(bf),
                "cos2": cos2,
                "sinsw2": sinsw2,
                "rot": rotm,
                "mtri": mtri2,
            }
        )
    return in_maps


def assemble_output(results, b, s, d):
    full = np.empty((b, s, d), np.float32)
    dout = d // 4
    for c in range(N_CORES):
        bb, g = divmod(c, 4)
        full[bb][:, dout * g : dout * (g + 1)] = results[c]["out"]
    return full


def kernel(**inputs):
    x = np.asarray(inputs["x"], np.float32)
    b, s, d = x.shape
    key = (s, d)
    if key not in _cache:
        _cache[key] = build_program(S=s, D=d)
    nc = _cache[key]
    in_maps = prep_inputs(
        x,
        np.asarray(inputs["cos"], np.float32),
        np.asarray(inputs["sin"], np.float32),
        np.asarray(inputs["wq"], np.float32),
        np.asarray(inputs["wk"], np.float32),
        np.asarray(inputs["wv"], np.float32),
        np.asarray(inputs["wo"], np.float32),
    )
    from concourse.bass_utils import run_bass_kernel_spmd

    res = run_bass_kernel_spmd(nc, in_maps, core_ids=list(range(N_CORES)))
    return assemble_output(res.results, b, s, d)
